# revision 12
# baseline (speedup 1.0000x reference)
"""APNB (asymmetric pyramid non-local block) sparse-attention kernel for 8 TRN2 NeuronCores.

Strategy: pure data-parallel over batch (B=8 -> one batch element per core, no
collectives). Per core, the whole block is computed with TensorE GEMMs
(f32 PSUM accumulation), bf16 on the accuracy-critical output path and
fp8-e4m3 DoubleRow (2x rate) on the attention path:

  host:        BN+bias folded into conv weights; W (value->out conv) is folded
               through the pooled value matrix on-device (WV2 = (Wc @ VT^T)^T),
               so the attention contribution to the output needs only a
               110-deep contraction instead of 256-deep ctx + 512x256 conv.
               X shipped twice: bf16 (output path) and fp8 (key path, 2x PE).
               Wk pre-scaled by 64 into fp8 range; un-scaled in the KF drain.
  phase 1 (streamed over blocks of 2 N-tiles):
               KF   = relu((Wk'*64 @ X8)/64 + bk')  fp8 DoubleRow matmuls
                                                    (256, 9216) persistent fp8
               OUTB = Bw' @ X                       bf16, (512, 9216) persistent
               col-pool partial sums of X and KF    (DVE reduce)
  phase 2:     row-pool + PSP pyramid (1,3,6,8) means -> PFX (X pools, bf16),
               KPX (KF pools, fp8)
               VT^T = Wv @ PFX                      (2x128, 110)
               WV2  = VT @ Wc^T                     (110, 512) via PE
  phase 3 (streamed over pairs of N-tiles):
               SIM^T = KP^T @ KF       one fp8 DoubleRow matmul per tile
               E1    = exp(s*SIM)                   (ACT)
               CSB   = ones_SxS @ E1   colsum broadcast to 110 partitions (PE)
               ATTN  = E1 * reciprocal(CSB)         (DVE, div-free softmax)
               OUT   = relu(OUTB + WV2^T @ ATTN + b')  (OUTB re-injected into
                       PSUM with an identity matmul; drains split ACT/DVE)
               OUT written bf16 (halves output DMA), upcast to f32 on host.

The softmax needs no max-subtraction: |s*sim| is O(1) for this problem's data
distribution, so exp is safe in f32. fp8 on the attention path is safe because
the attention context contributes only a few percent of the output magnitude.
"""

import numpy as np
import ml_dtypes
from contextlib import ExitStack

import concourse.bass as bass
import concourse.bacc as bacc
import concourse.mybir as mybir
import concourse.tile as tile
from concourse.bass import ts, ds
from concourse.bass_utils import run_bass_kernel_spmd
from concourse.masks import make_identity

P = 128
CIN, CK, CV, COUT = 512, 256, 256, 512
H = W = 96
N = H * W              # 9216
NT = 512               # matmul free-dim tile
NTILES = N // NT       # 18
NB = 2                 # tiles per phase-1 block
S = 110                # pooled tokens: 1+9+36+64
SP = 112               # padded S so the fp8 DoubleRow k-group step is 16-aligned
EPS = 1e-5
WKSCALE = 64.0         # Wk prescale into fp8-normal range
F32 = mybir.dt.float32
BF16 = mybir.dt.bfloat16
FP8 = mybir.dt.float8e4
RELU = mybir.ActivationFunctionType.Relu
EXP = mybir.ActivationFunctionType.Exp
COPY = mybir.ActivationFunctionType.Copy
ADD = mybir.AluOpType.add
MAX = mybir.AluOpType.max
DR = mybir.MatmulPerfMode.DoubleRow
AX = mybir.AxisListType

N_CORES = 8

# set by test harness to capture profile info
PROFILE = False
LAST_RESULT = None

_NC = None


def _psp_stage2(nc, pool, g24, ko, pf):
    """g24: [P, ko, 24, 24] 4x4-pixel SUMS. pf: [P, ko, 110+] pyramid means.

    Reference concat order (1, 3, 6, 8). Scale-6 cells are 4x4 grid cells;
    scale-8 are 3x3; scale-3 = 2x2 of scale-6; scale-1 = sum of all scale-3.
    """
    f = F32
    lp = nc.allow_low_precision
    # ---- scale 6 (cells of 4x4 grid entries = 16x16 px) ----
    c6 = pool.tile([P, ko, 24, 6], BF16, tag="c6")
    with lp(reason="pool partials"):
        nc.vector.reduce_sum(c6, g24.rearrange("p k a (b bi) -> p k a b bi", bi=4), axis=AX.X)
    v6 = c6.rearrange("p k (a ai) b -> p k a ai b", ai=4)
    s6 = pool.tile([P, ko, 6, 6], f, tag="s6")
    nc.vector.tensor_add(s6, v6[:, :, :, 0, :], v6[:, :, :, 1, :])
    nc.vector.tensor_add(s6, s6, v6[:, :, :, 2, :])
    nc.vector.tensor_add(s6, s6, v6[:, :, :, 3, :])
    with lp(reason="pyramid means"):
        nc.vector.tensor_scalar_mul(
            pf[:, :, 10:46], s6.rearrange("p k a b -> p k (a b)"), 1.0 / 256.0
        )
    # ---- scale 3 (2x2 of scale-6 cells = 32x32 px) ----
    c3 = pool.tile([P, ko, 6, 3], f, tag="c3")
    nc.vector.reduce_sum(c3, s6.rearrange("p k a (b bi) -> p k a b bi", bi=2), axis=AX.X)
    v3 = c3.rearrange("p k (a ai) b -> p k a ai b", ai=2)
    s3 = pool.tile([P, ko, 3, 3], f, tag="s3")
    nc.vector.tensor_add(s3, v3[:, :, :, 0, :], v3[:, :, :, 1, :])
    with lp(reason="pyramid means"):
        nc.vector.tensor_scalar_mul(
            pf[:, :, 1:10], s3.rearrange("p k a b -> p k (a b)"), 1.0 / 1024.0
        )
    # ---- scale 1 ----
    t1 = pool.tile([P, ko, 1], f, tag="t1")
    nc.vector.reduce_sum(t1, s3.rearrange("p k a b -> p k (a b)"), axis=AX.X)
    with lp(reason="pyramid means"):
        nc.vector.tensor_scalar_mul(pf[:, :, 0:1], t1, 1.0 / 9216.0)
    # ---- scale 8 (cells of 3x3 grid entries = 12x12 px) ----
    c8 = pool.tile([P, ko, 24, 8], BF16, tag="c8")
    with lp(reason="pool partials"):
        nc.vector.reduce_sum(c8, g24.rearrange("p k a (b bi) -> p k a b bi", bi=3), axis=AX.X)
    v8 = c8.rearrange("p k (a ai) b -> p k a ai b", ai=3)
    s8 = pool.tile([P, ko, 8, 8], f, tag="s8")
    nc.vector.tensor_add(s8, v8[:, :, :, 0, :], v8[:, :, :, 1, :])
    nc.vector.tensor_add(s8, s8, v8[:, :, :, 2, :])
    with lp(reason="pyramid means"):
        nc.vector.tensor_scalar_mul(
            pf[:, :, 46:110], s8.rearrange("p k a b -> p k (a b)"), 1.0 / 144.0
        )


def _build_body(ctx: ExitStack, tc: tile.TileContext, x_d, x8_d, wk8_d, wvt_d,
                wct_d, bwt_d, bk_d, bf_d, out_d):
    nc = tc.nc

    consts = ctx.enter_context(tc.tile_pool(name="consts", bufs=1))
    big = ctx.enter_context(tc.tile_pool(name="big", bufs=1))
    stage = ctx.enter_context(tc.tile_pool(name="stage", bufs=2))
    poolb = ctx.enter_context(tc.tile_pool(name="poolb", bufs=1))
    work = ctx.enter_context(tc.tile_pool(name="work", bufs=4))
    outp = ctx.enter_context(tc.tile_pool(name="outp", bufs=3))

    # ---- phase-1-critical weights first so the first matmul isn't stuck
    # behind phase-2/3 constants on the DMA queue (all weight params are
    # host-packed [P, ...] so each partition is one contiguous descriptor) ----
    wk8 = consts.tile([P, 4, CK], FP8)
    nc.sync.dma_start(wk8, wk8_d[:])
    bkb = consts.tile([P, 2], F32)
    nc.sync.dma_start(bkb, bk_d[:])
    bwt = consts.tile([P, 4, COUT], BF16)

    # ---- persistent full-res activations ----
    kfb = big.tile([P, 2, N], FP8)     # relu(key/query features)
    outb = big.tile([P, 4, N], BF16)   # Bw' @ X partial of the output conv
    gcolx = poolb.tile([P, 4, 2304], BF16)  # X col-pool sums (4-px groups)
    g24k = poolb.tile([P, 2, 24, 24], F32)  # KF 4x4-block sums (24x24 grid)

    xv = x_d[:].rearrange("(kc p) n -> p kc n", p=P)
    x8v = x8_d[:].rearrange("(kc p) n -> p kc n", p=P)
    ov = out_d[:].rearrange("(mc p) n -> p mc n", p=P)
    kg = kfb.rearrange("p k (h w) -> p k h w", w=W)
    hc_done = 0

    # ---- phase 1: stream X in blocks of NB tiles; KF (fp8 DoubleRow),
    # OUTB (bf16), col-pools. Stationary-major loop order so one LDWEIGHTS
    # serves NB matmuls. ----
    nblocks = NTILES // NB
    # phase-2/3 consts, declared early so their DMAs overlap phase-1 compute
    # (issued after the first block's X DMAs below)
    wvt = consts.tile([P, 4, CV], BF16)
    wct = consts.tile([P, 2, COUT], BF16)
    bfb = consts.tile([P, 4], F32)
    g24x = poolb.tile([P, 4, 24, 24], BF16)
    gxv = gcolx.rearrange("p k (hb hi wb) -> p k hb hi wb", hi=4, wb=24)
    with tc.tile_pool(name="psA", bufs=2, space="PSUM") as psA:
        for b in range(nblocks):
            c0 = b * NB * NT
            x8t = stage.tile([P, 4, NB * NT], FP8, tag="x8t")
            xt = stage.tile([P, 4, NB * NT], BF16, tag="xt")
            for j in range(NB):
                nc.sync.dma_start(x8t[:, :, ts(j, NT)], x8v[:, :, ds(c0 + j * NT, NT)])
            for j in range(NB):
                nc.sync.dma_start(xt[:, :, ts(j, NT)], xv[:, :, ds(c0 + j * NT, NT)])
            if b == 0:
                nc.sync.dma_start(bwt, bwt_d[:])
            elif b == 1:
                nc.sync.dma_start(wvt, wvt_d[:])
                nc.sync.dma_start(wct, wct_d[:])
                nc.sync.dma_start(bfb, bf_d[:])
            def kf_stage():
                # KF: fp8 DoubleRow, contraction 512 = 2 k-pair groups
                for mc in range(2):
                    ps = psA.tile([P, NB, NT], F32, tag="kf", name=f"kps{b}_{mc}")
                    for k2 in range(2):
                        for j in range(NB):
                            nc.tensor.matmul(ps[:, j, :],
                                             wk8[:, 2 * k2:2 * k2 + 2, ts(mc, P)],
                                             x8t[:, 2 * k2:2 * k2 + 2, ts(j, NT)],
                                             start=(k2 == 0), stop=(k2 == 1),
                                             perf_mode=DR)
                    nc.scalar.activation(kfb[:, mc, ds(c0, NB * NT)],
                                         ps.rearrange("p j n -> p (j n)"), RELU,
                                         bias=bkb[:, mc:mc + 1], scale=1.0 / WKSCALE)

            def outb_stage():
                # OUTB: bf16, mc 0..3
                for mc in range(4):
                    ps = psA.tile([P, NB, NT], F32, tag="ob", name=f"ops{b}_{mc}")
                    for kc in range(4):
                        for j in range(NB):
                            nc.tensor.matmul(ps[:, j, :], bwt[:, kc, ts(mc, P)],
                                             xt[:, kc, ts(j, NT)],
                                             start=(kc == 0), stop=(kc == 3))
                    nc.scalar.activation(outb[:, mc, ds(c0, NB * NT)],
                                         ps.rearrange("p j n -> p (j n)"), COPY)

            # last block: OUTB first, so the KF-pool -> pyramid chain that
            # gates phase 3 overlaps the OUTB matmul stream
            if b == nblocks - 1:
                outb_stage()
                kf_stage()
            else:
                kf_stage()
                outb_stage()
            # X col-pool for this block (4-px groups along w). Stage 1 is a
            # packed pair-add on DVE (16-bit 2x mode: (x0+x2, x1+x3)); stage 2
            # adds the interleaved pair on idle GpSimd.
            ng = NB * P  # 4-px groups in this block
            xg = xt.rearrange("p k (g gi) -> p k g gi", gi=4)
            gct = stage.tile([P, 4, ng, 2], BF16, tag="gct")
            with nc.allow_low_precision(reason="pyramid-pool partials in bf16"):
                nc.vector.tensor_add(gct, xg[:, :, :, 0:2], xg[:, :, :, 2:4])
                nc.gpsimd.tensor_add(gcolx[:, :, ds(b * ng, ng)],
                                     gct[:, :, :, 0], gct[:, :, :, 1])
            # KF pooling straight to the 24x24 grid, in 12-row chunks (kfb is
            # persistent, so chunks can span tile boundaries)
            while hc_done < 8 and (hc_done + 1) * 1152 <= c0 + NB * NT:
                hc = hc_done
                for k in range(2):
                    src_ap = kg[:, k, ts(hc, 12), :].rearrange(
                        "p (hb hi) (wb wi) -> p hb wb hi wi", hi=4, wi=4)
                    nc.vector.reduce_sum(g24k[:, k, ts(hc, 3), :], src_ap,
                                         axis=AX.XY)
                hc_done += 1
            # X row-pool onto the 24x24 grid, in 8-row-of-grid chunks
            # (8 grid rows = 32 px rows = 3072 px = 3 blocks)
            if b % 3 == 2:
                hb0 = (b // 3) * 8
                sl = (slice(None), slice(None), ds(hb0, 8))
                nc.vector.tensor_add(g24x[sl], gxv[sl + (0,)], gxv[sl + (1,)])
                nc.vector.tensor_add(g24x[sl], g24x[sl], gxv[sl + (2,)])
                nc.vector.tensor_add(g24x[sl], g24x[sl], gxv[sl + (3,)])

    # ---- small on-chip constants ----
    ones_sxs = consts.tile([S, S], BF16)
    nc.vector.memset(ones_sxs, 1.0)
    ident = consts.tile([P, P], BF16)
    make_identity(nc, ident)

    with tc.tile_pool(name="psB", bufs=2, space="PSUM") as psB, \
         tc.tile_pool(name="psO", bufs=2, space="PSUM") as psO:
        # ---- phase 2: pyramid means, VT^T, WV2 ----
        pfx = consts.tile([P, 4, S], BF16)
        _psp_stage2(nc, poolb, g24x, 4, pfx)
        kpx = consts.tile([P, 2, SP], FP8)
        _psp_stage2(nc, poolb, g24k, 2, kpx)

        # VT^T = Wv @ PFX : [2*128 (v), 110 (s)]
        vtT = consts.tile([P, 2, S], BF16)
        for vc in range(2):
            vtT_ps = psB.tile([P, S], F32, tag="sim", name=f"vtT_ps{vc}")
            for kc in range(4):
                nc.tensor.matmul(vtT_ps, wvt[:, kc, ts(vc, P)], pfx[:, kc, :],
                                 start=(kc == 0), stop=(kc == 3))
            nc.scalar.copy(vtT[:, vc, :], vtT_ps)

        # WV2 = VT @ Wc^T : [110 (s), 512 (out)] -- stationary for the
        # attention->output matmul (folds the value->out conv into VT)
        wv2_ps = psB.tile([P, NT], F32, tag="csb", name="wv2_ps")
        for vc in range(2):
            nc.tensor.matmul(wv2_ps[:S, :], vtT[:, vc, :], wct[:, vc, :],
                             start=(vc == 0), stop=(vc == 1))
        wv2 = consts.tile([S, COUT], BF16)
        nc.scalar.copy(wv2, wv2_ps[:S, :])

        # ---- phase 3: attention + output, software-pipelined over pairs of
        # N tiles: pair k's softmax chain (ACT exp -> PE colsum -> DVE recip
        # -> GpSimd mult) fills while pair k-1's 16 output matmuls stream. ----
        def out_stage(tt, ens):
            for mc in range(4):
                ops = psO.tile([P, 2, NT], F32, tag="outp", name=f"ops{tt[0]}_{mc}")
                nc.tensor.matmul(ops[:, 0, :], ident, outb[:, mc, ts(tt[0], NT)],
                                 start=True, stop=False)
                nc.tensor.matmul(ops[:, 1, :], ident, outb[:, mc, ts(tt[1], NT)],
                                 start=True, stop=False)
                nc.tensor.matmul(ops[:, 0, :], wv2[:, ts(mc, P)], ens[0][:S, :],
                                 start=False, stop=True)
                nc.tensor.matmul(ops[:, 1, :], wv2[:, ts(mc, P)], ens[1][:S, :],
                                 start=False, stop=True)
                osb = outp.tile([P, 2, NT], BF16, tag="osb")
                opsv = ops.rearrange("p j n -> p (j n)")
                osv = osb.rearrange("p j n -> p (j n)")
                if mc < 2:
                    nc.scalar.activation(osv, opsv, RELU, bias=bfb[:, mc:mc + 1])
                else:
                    with nc.allow_low_precision(reason="output bf16"):
                        nc.vector.tensor_scalar(osv, opsv,
                                                scalar1=bfb[:, mc:mc + 1],
                                                scalar2=0.0, op0=ADD, op1=MAX)
                nc.sync.dma_start(ov[:, mc, ds(tt[0] * NT, 2 * NT)], osv)

        prev = None
        for tp in range(NTILES // 2):
            tt = (2 * tp, 2 * tp + 1)
            sims = []
            for t in tt:
                sim_ps = psB.tile([P, NT], F32, tag="sim")
                nc.tensor.matmul(sim_ps[:S, :], kpx[:, 0:2, 0:S],
                                 kfb[:, 0:2, ds(t * NT, NT)],
                                 start=True, stop=True, perf_mode=DR)
                sims.append(sim_ps)
            e1s = []
            for sim_ps in sims:
                e1 = work.tile([P, NT], BF16, tag="e1")
                nc.scalar.activation(e1[:S, :], sim_ps[:S, :], EXP, scale=0.0625)
                e1s.append(e1)
            if prev is not None:
                out_stage(*prev)
            ens = []
            for e1 in e1s:
                # colsum broadcast to all 110 partitions in one matmul
                csb_ps = psB.tile([P, NT], F32, tag="csb")
                nc.tensor.matmul(csb_ps[:S, :], ones_sxs, e1[:S, :],
                                 start=True, stop=True)
                rcol = work.tile([P, NT], F32, tag="rcol")
                nc.vector.reciprocal_approx_fast(rcol[:S, :], csb_ps[:S, :])
                en = work.tile([P, NT], BF16, tag="en")
                with nc.allow_low_precision(reason="softmax norm"):
                    nc.gpsimd.tensor_mul(en[:S, :], e1[:S, :], rcol[:S, :])
                ens.append(en)
            prev = (tt, ens)
        out_stage(*prev)


def _prune_redundant_ldweights(nc):
    """Remove back-to-back InstLdweights that reload the exact same stationary
    operand (walrus emits one LDWEIGHTS per matmul; our stationary-major loop
    order makes most of them redundant, and dropping them lets consecutive
    matmuls pipeline their fill/drain). All stationaries in this kernel are
    written exactly once before first use, so a signature match is sufficient.
    """
    n_pruned = 0
    for f in nc.m.functions:
        for blk in f.blocks:
            insts = list(blk.instructions)
            out = []
            last_sig = None
            for inst in insts:
                tname = type(inst).__name__
                eng = getattr(inst, "engine", None)
                if eng == mybir.EngineType.PE:
                    if tname == "InstLdweights":
                        ap = inst.ins[0]
                        sig = (ap.memref, ap.offset, str(ap.ap), str(ap.dtype),
                               str(inst.perf_mode), str(inst.is_transpose),
                               str(inst.tile_position), str(inst.tile_size))
                        if sig == last_sig and inst.sync_info is None:
                            n_pruned += 1
                            continue
                        last_sig = sig
                    elif tname in ("InstMatmult", "InstEventSemaphore"):
                        pass
                    else:
                        last_sig = None
                out.append(inst)
            blk.instructions = out
    return n_pruned


def _patch_act_tables():
    """Force every activation onto the one table that holds Exp, Relu and
    Copy together (`natural_log_exp_and_others`), so the kernel does a single
    ACT_TABLE_LOAD instead of reloading on table switches.

    Table ids are positional (index into act_info.json), so we keep the dict
    order/size and just empty the other entries.
    """
    import concourse.hw_specs as hw_specs

    if getattr(bacc, "_apnb_act_patch", False):
        return
    orig = hw_specs.get_activation_tables

    def patched(module_arch):
        tabs = orig(module_arch)
        keep = "natural_log_exp_and_others"
        if keep not in tabs:
            return tabs
        return {k: (v if k == keep else set()) for k, v in tabs.items()}

    bacc.get_activation_tables = patched
    bacc._apnb_act_patch = True


def build_nc():
    _patch_act_tables()
    nc = bacc.Bacc("TRN2", target_bir_lowering=False, debug=False)
    x_d = nc.declare_dram_parameter("x", [CIN, N], BF16, isOutput=False)
    x8_d = nc.declare_dram_parameter("x8", [CIN, N], FP8, isOutput=False)
    wk8_d = nc.declare_dram_parameter("wk8", [P, 4 * CK], FP8, isOutput=False)
    wvt_d = nc.declare_dram_parameter("wvt", [P, 4 * CV], BF16, isOutput=False)
    wct_d = nc.declare_dram_parameter("wct", [P, 2 * COUT], BF16, isOutput=False)
    bwt_d = nc.declare_dram_parameter("bwt", [P, 4 * COUT], BF16, isOutput=False)
    bk_d = nc.declare_dram_parameter("bk", [P, 2], F32, isOutput=False)
    bf_d = nc.declare_dram_parameter("bf", [P, 4], F32, isOutput=False)
    out_d = nc.declare_dram_parameter("out", [COUT, N], BF16, isOutput=True)
    with tile.TileContext(nc) as tc:
        with ExitStack() as ctx:
            _build_body(ctx, tc, x_d, x8_d, wk8_d, wvt_d, wct_d, bwt_d, bk_d,
                        bf_d, out_d)
    nc.compile()
    _prune_redundant_ldweights(nc)
    return nc


def _get_nc():
    global _NC
    if _NC is None:
        _NC = build_nc()
    return _NC


def fold_params(Wk, bk, gk, betak, mk, vk, Wv, bv, Ww, bw, Wo, bo, go, betao,
                mo, vo):
    """Fold BN params + the Ww conv into effective weights (all f32 numpy)."""
    bf16 = ml_dtypes.bfloat16
    fp8 = ml_dtypes.float8_e4m3
    sk = gk / np.sqrt(vk + EPS)
    Wk_f = sk[:, None] * Wk
    bk_f = (bk - mk) * sk + betak
    so = go / np.sqrt(vo + EPS)
    A = so[:, None] * Wo[:, :CIN]      # applies to ctx2 = Ww@ctx + bw
    Bw = so[:, None] * Wo[:, CIN:]     # applies to feats
    b0 = (bo - mo) * so + betao
    Wc = A @ Ww                        # (COUT, CV)
    # attn rows sum to 1  =>  value bias bv contributes Wc @ bv everywhere
    bf_ = b0 + A @ bw + Wc @ bv
    def pack(Wt):
        # [kc*128+p, m] -> [p, kc*M+m]: one contiguous DMA row per partition
        kc = Wt.shape[0] // P
        return np.ascontiguousarray(
            Wt.reshape(kc, P, -1).transpose(1, 0, 2).reshape(P, -1))

    return {
        "wk8": pack(Wk_f.T * WKSCALE).astype(fp8),
        "wvt": pack(Wv.T).astype(bf16),
        "wct": pack(Wc.T).astype(bf16),
        "bwt": pack(Bw.T).astype(bf16),
        "bk": pack(bk_f[:, None]).astype(np.float32),
        "bf": pack(bf_[:, None]).astype(np.float32),
    }


def kernel(**inputs):
    global LAST_RESULT
    feats = np.asarray(inputs["feats"], np.float32)
    B = feats.shape[0]
    assert feats.shape == (B, CIN, H, W) and B == N_CORES

    common = fold_params(
        np.asarray(inputs["Wk"], np.float32), np.asarray(inputs["bk"], np.float32),
        np.asarray(inputs["gk"], np.float32), np.asarray(inputs["betak"], np.float32),
        np.asarray(inputs["mk"], np.float32), np.asarray(inputs["vk"], np.float32),
        np.asarray(inputs["Wv"], np.float32), np.asarray(inputs["bv"], np.float32),
        np.asarray(inputs["Ww"], np.float32), np.asarray(inputs["bw"], np.float32),
        np.asarray(inputs["Wo"], np.float32), np.asarray(inputs["bo"], np.float32),
        np.asarray(inputs["go"], np.float32), np.asarray(inputs["betao"], np.float32),
        np.asarray(inputs["mo"], np.float32), np.asarray(inputs["vo"], np.float32),
    )
    bf16 = ml_dtypes.bfloat16
    fp8 = ml_dtypes.float8_e4m3
    in_maps = []
    for i in range(N_CORES):
        xi = np.ascontiguousarray(feats[i].reshape(CIN, N))
        in_maps.append({"x": xi.astype(bf16), "x8": xi.astype(fp8), **common})
    nc = _get_nc()
    res = run_bass_kernel_spmd(nc, in_maps, core_ids=list(range(N_CORES)),
                               trace=PROFILE)
    LAST_RESULT = res
    out = np.stack([res.results[i]["out"].astype(np.float32).reshape(COUT, H, W)
                    for i in range(N_CORES)])
    return out


# revision 14
# speedup vs baseline: 1.0124x; 1.0124x over previous
"""APNB (asymmetric pyramid non-local block) sparse-attention kernel for 8 TRN2 NeuronCores.

Strategy: pure data-parallel over batch (B=8 -> one batch element per core, no
collectives). Per core, the whole block is computed with TensorE GEMMs
(f32 PSUM accumulation), bf16 on the accuracy-critical output path and
fp8-e4m3 DoubleRow (2x rate) on the attention path:

  host:        BN+bias folded into conv weights; W (value->out conv) is folded
               through the pooled value matrix on-device (WV2 = (Wc @ VT^T)^T),
               so the attention contribution to the output needs only a
               110-deep contraction instead of 256-deep ctx + 512x256 conv.
               X shipped twice: bf16 (output path) and fp8 (key path, 2x PE).
               Wk pre-scaled by 64 into fp8 range; un-scaled in the KF drain.
  phase 1 (streamed over blocks of 2 N-tiles):
               KF   = relu((Wk'*64 @ X8)/64 + bk')  fp8 DoubleRow matmuls
                                                    (256, 9216) persistent fp8
               OUTB = Bw' @ X                       bf16, (512, 9216) persistent
               col-pool partial sums of X and KF    (DVE reduce)
  phase 2:     row-pool + PSP pyramid (1,3,6,8) means -> PFX (X pools, bf16),
               KPX (KF pools, fp8)
               VT^T = Wv @ PFX                      (2x128, 110)
               WV2  = VT @ Wc^T                     (110, 512) via PE
  phase 3 (streamed over pairs of N-tiles):
               SIM^T = KP^T @ KF       one fp8 DoubleRow matmul per tile
               E1    = exp(s*SIM)                   (ACT)
               CSB   = ones_SxS @ E1   colsum broadcast to 110 partitions (PE)
               ATTN  = E1 * reciprocal(CSB)         (DVE, div-free softmax)
               OUT   = relu(OUTB + WV2^T @ ATTN + b')  (OUTB re-injected into
                       PSUM with an identity matmul; drains split ACT/DVE)
               OUT written bf16 (halves output DMA), upcast to f32 on host.

The softmax needs no max-subtraction: |s*sim| is O(1) for this problem's data
distribution, so exp is safe in f32. fp8 on the attention path is safe because
the attention context contributes only a few percent of the output magnitude.
"""

import numpy as np
import ml_dtypes
from contextlib import ExitStack

import concourse.bass as bass
import concourse.bacc as bacc
import concourse.mybir as mybir
import concourse.tile as tile
from concourse.bass import ts, ds
from concourse.bass_utils import run_bass_kernel_spmd
from concourse.masks import make_identity

P = 128
CIN, CK, CV, COUT = 512, 256, 256, 512
H = W = 96
N = H * W              # 9216
NT = 512               # matmul free-dim tile
NTILES = N // NT       # 18
NB = 2                 # tiles per phase-1 block
S = 110                # pooled tokens: 1+9+36+64
SP = 112               # padded S so the fp8 DoubleRow k-group step is 16-aligned
EPS = 1e-5
WKSCALE = 64.0         # Wk prescale into fp8-normal range
F32 = mybir.dt.float32
BF16 = mybir.dt.bfloat16
FP8 = mybir.dt.float8e4
RELU = mybir.ActivationFunctionType.Relu
EXP = mybir.ActivationFunctionType.Exp
COPY = mybir.ActivationFunctionType.Copy
ADD = mybir.AluOpType.add
MAX = mybir.AluOpType.max
DR = mybir.MatmulPerfMode.DoubleRow
AX = mybir.AxisListType

N_CORES = 8

# set by test harness to capture profile info
PROFILE = False
LAST_RESULT = None

_NC = None


def _psp_stage2(nc, pool, g24, ko, pf):
    """g24: [P, ko, 24, 24] 4x4-pixel SUMS. pf: [P, ko, 110+] pyramid means.

    Reference concat order (1, 3, 6, 8). Scale-6 cells are 4x4 grid cells;
    scale-8 are 3x3; scale-3 = 2x2 of scale-6; scale-1 = sum of all scale-3.
    """
    f = F32
    lp = nc.allow_low_precision
    # ---- scale 6 (cells of 4x4 grid entries = 16x16 px) ----
    c6 = pool.tile([P, ko, 24, 6], BF16, tag="c6")
    with lp(reason="pool partials"):
        nc.vector.reduce_sum(c6, g24.rearrange("p k a (b bi) -> p k a b bi", bi=4), axis=AX.X)
    v6 = c6.rearrange("p k (a ai) b -> p k a ai b", ai=4)
    s6 = pool.tile([P, ko, 6, 6], f, tag="s6")
    nc.vector.tensor_add(s6, v6[:, :, :, 0, :], v6[:, :, :, 1, :])
    nc.vector.tensor_add(s6, s6, v6[:, :, :, 2, :])
    nc.vector.tensor_add(s6, s6, v6[:, :, :, 3, :])
    with lp(reason="pyramid means"):
        nc.vector.tensor_scalar_mul(
            pf[:, :, 10:46], s6.rearrange("p k a b -> p k (a b)"), 1.0 / 256.0
        )
    # ---- scale 3 (2x2 of scale-6 cells = 32x32 px) ----
    c3 = pool.tile([P, ko, 6, 3], f, tag="c3")
    nc.vector.reduce_sum(c3, s6.rearrange("p k a (b bi) -> p k a b bi", bi=2), axis=AX.X)
    v3 = c3.rearrange("p k (a ai) b -> p k a ai b", ai=2)
    s3 = pool.tile([P, ko, 3, 3], f, tag="s3")
    nc.vector.tensor_add(s3, v3[:, :, :, 0, :], v3[:, :, :, 1, :])
    with lp(reason="pyramid means"):
        nc.vector.tensor_scalar_mul(
            pf[:, :, 1:10], s3.rearrange("p k a b -> p k (a b)"), 1.0 / 1024.0
        )
    # ---- scale 1 ----
    t1 = pool.tile([P, ko, 1], f, tag="t1")
    nc.vector.reduce_sum(t1, s3.rearrange("p k a b -> p k (a b)"), axis=AX.X)
    with lp(reason="pyramid means"):
        nc.vector.tensor_scalar_mul(pf[:, :, 0:1], t1, 1.0 / 9216.0)
    # ---- scale 8 (cells of 3x3 grid entries = 12x12 px) ----
    c8 = pool.tile([P, ko, 24, 8], BF16, tag="c8")
    with lp(reason="pool partials"):
        nc.vector.reduce_sum(c8, g24.rearrange("p k a (b bi) -> p k a b bi", bi=3), axis=AX.X)
    v8 = c8.rearrange("p k (a ai) b -> p k a ai b", ai=3)
    s8 = pool.tile([P, ko, 8, 8], f, tag="s8")
    nc.vector.tensor_add(s8, v8[:, :, :, 0, :], v8[:, :, :, 1, :])
    nc.vector.tensor_add(s8, s8, v8[:, :, :, 2, :])
    with lp(reason="pyramid means"):
        nc.vector.tensor_scalar_mul(
            pf[:, :, 46:110], s8.rearrange("p k a b -> p k (a b)"), 1.0 / 144.0
        )


def _build_body(ctx: ExitStack, tc: tile.TileContext, x_d, x8_d, wk8_d, wvt_d,
                wct_d, bwt_d, bk_d, bf_d, out_d):
    nc = tc.nc

    consts = ctx.enter_context(tc.tile_pool(name="consts", bufs=1))
    big = ctx.enter_context(tc.tile_pool(name="big", bufs=1))
    stage = ctx.enter_context(tc.tile_pool(name="stage", bufs=2))
    poolb = ctx.enter_context(tc.tile_pool(name="poolb", bufs=1))
    work = ctx.enter_context(tc.tile_pool(name="work", bufs=4))
    outp = ctx.enter_context(tc.tile_pool(name="outp", bufs=3))

    # ---- phase-1-critical weights first so the first matmul isn't stuck
    # behind phase-2/3 constants on the DMA queue (all weight params are
    # host-packed [P, ...] so each partition is one contiguous descriptor) ----
    wk8 = consts.tile([P, 4, CK], FP8)
    nc.sync.dma_start(wk8, wk8_d[:])
    bkb = consts.tile([P, 2], F32)
    nc.sync.dma_start(bkb, bk_d[:])
    bwt = consts.tile([P, 4, COUT], BF16)

    # ---- persistent full-res activations ----
    kfb = big.tile([P, 2, N], FP8)     # relu(key/query features)
    outb = big.tile([P, 4, N], BF16)   # Bw' @ X partial of the output conv
    gcolx = poolb.tile([P, 4, 2304], BF16)  # X col-pool sums (4-px groups)
    g24k = poolb.tile([P, 2, 24, 24], F32)  # KF 4x4-block sums (24x24 grid)

    xv = x_d[:].rearrange("(kc p) n -> p kc n", p=P)
    x8v = x8_d[:].rearrange("(kc p) n -> p kc n", p=P)
    ov = out_d[:].rearrange("(mc p) n -> p mc n", p=P)
    kg = kfb.rearrange("p k (h w) -> p k h w", w=W)
    hc_done = 0

    # ---- phase 1: stream X in blocks of NB tiles; KF (fp8 DoubleRow),
    # OUTB (bf16), col-pools. Stationary-major loop order so one LDWEIGHTS
    # serves NB matmuls. ----
    nblocks = NTILES // NB
    # phase-2/3 consts, declared early so their DMAs overlap phase-1 compute
    # (issued after the first block's X DMAs below)
    wvt = consts.tile([P, 4, CV], BF16)
    wct = consts.tile([P, 2, COUT], BF16)
    bfb = consts.tile([P, 4], F32)
    g24x = poolb.tile([P, 4, 24, 24], BF16)
    gxv = gcolx.rearrange("p k (hb hi wb) -> p k hb hi wb", hi=4, wb=24)
    def kf_stage(psA, b, x8t):
        # KF: fp8 DoubleRow, contraction 512 = 2 k-pair groups
        c0 = b * NB * NT
        for mc in range(2):
            ps = psA.tile([P, NB, NT], F32, tag="kf", name=f"kps{b}_{mc}")
            for k2 in range(2):
                for j in range(NB):
                    nc.tensor.matmul(ps[:, j, :],
                                     wk8[:, 2 * k2:2 * k2 + 2, ts(mc, P)],
                                     x8t[:, 2 * k2:2 * k2 + 2, ts(j, NT)],
                                     start=(k2 == 0), stop=(k2 == 1),
                                     perf_mode=DR)
            nc.scalar.activation(kfb[:, mc, ds(c0, NB * NT)],
                                 ps.rearrange("p j n -> p (j n)"), RELU,
                                 bias=bkb[:, mc:mc + 1], scale=1.0 / WKSCALE)
        # KF pooling straight to the 24x24 grid, in 12-row chunks (kfb is
        # persistent, so chunks can span tile boundaries)
        nonlocal hc_done
        while hc_done < 8 and (hc_done + 1) * 1152 <= c0 + NB * NT:
            hc = hc_done
            for k in range(2):
                src_ap = kg[:, k, ts(hc, 12), :].rearrange(
                    "p (hb hi) (wb wi) -> p hb wb hi wi", hi=4, wi=4)
                nc.vector.reduce_sum(g24k[:, k, ts(hc, 3), :], src_ap,
                                     axis=AX.XY)
            hc_done += 1

    def outb_stage(psA, b, xt):
        # OUTB: bf16, mc 0..3
        c0 = b * NB * NT
        for mc in range(4):
            ps = psA.tile([P, NB, NT], F32, tag="ob", name=f"obs{b}_{mc}")
            for kc in range(4):
                for j in range(NB):
                    nc.tensor.matmul(ps[:, j, :], bwt[:, kc, ts(mc, P)],
                                     xt[:, kc, ts(j, NT)],
                                     start=(kc == 0), stop=(kc == 3))
            nc.scalar.activation(outb[:, mc, ds(c0, NB * NT)],
                                 ps.rearrange("p j n -> p (j n)"), COPY)

    def xpool_stage(b, xt):
        # X col-pool for this block (4-px groups along w). Stage 1 is a
        # packed pair-add on DVE (16-bit 2x mode: (x0+x2, x1+x3)); stage 2
        # adds the interleaved pair on idle GpSimd.
        ng = NB * P  # 4-px groups in this block
        xg = xt.rearrange("p k (g gi) -> p k g gi", gi=4)
        gct = stage.tile([P, 4, ng, 2], BF16, tag="gct", name=f"gct{b}")
        with nc.allow_low_precision(reason="pyramid-pool partials in bf16"):
            nc.vector.tensor_add(gct, xg[:, :, :, 0:2], xg[:, :, :, 2:4])
            nc.gpsimd.tensor_add(gcolx[:, :, ds(b * ng, ng)],
                                 gct[:, :, :, 0], gct[:, :, :, 1])
        # X row-pool onto the 24x24 grid, in 8-row-of-grid chunks
        # (8 grid rows = 32 px rows = 3072 px = 3 blocks)
        if b % 3 == 2:
            hb0 = (b // 3) * 8
            sl = (slice(None), slice(None), ds(hb0, 8))
            nc.vector.tensor_add(g24x[sl], gxv[sl + (0,)], gxv[sl + (1,)])
            nc.vector.tensor_add(g24x[sl], g24x[sl], gxv[sl + (2,)])
            nc.vector.tensor_add(g24x[sl], g24x[sl], gxv[sl + (3,)])

    # Phase-1 software pipeline: KF for block b runs one block ahead of OUTB
    # for block b-1, so the final KF-pool -> pyramid chain overlaps the last
    # OUTB matmul stream and the startup only waits for x8 (not xt+bwt).
    with tc.tile_pool(name="psA", bufs=2, space="PSUM") as psA:
        prev_xt = None
        for b in range(nblocks):
            c0 = b * NB * NT
            x8t = stage.tile([P, 4, NB * NT], FP8, tag="x8t", name=f"x8t{b}")
            for j in range(NB):
                nc.sync.dma_start(x8t[:, :, ts(j, NT)], x8v[:, :, ds(c0 + j * NT, NT)])
            if b == 0:
                nc.sync.dma_start(bwt, bwt_d[:])
            xt = stage.tile([P, 4, NB * NT], BF16, tag="xt", name=f"xt{b}")
            for j in range(NB):
                nc.sync.dma_start(xt[:, :, ts(j, NT)], xv[:, :, ds(c0 + j * NT, NT)])
            if b == 1:
                nc.sync.dma_start(wvt, wvt_d[:])
                nc.sync.dma_start(wct, wct_d[:])
                nc.sync.dma_start(bfb, bf_d[:])
            kf_stage(psA, b, x8t)
            if prev_xt is not None:
                outb_stage(psA, b - 1, prev_xt)
            prev_xt = xt
        outb_stage(psA, nblocks - 1, prev_xt)

    # ---- small on-chip constants ----
    ones_sxs = consts.tile([S, S], BF16)
    nc.vector.memset(ones_sxs, 1.0)
    ident = consts.tile([P, P], BF16)
    make_identity(nc, ident)

    with tc.tile_pool(name="psB", bufs=2, space="PSUM") as psB, \
         tc.tile_pool(name="psO", bufs=2, space="PSUM") as psO:
        # ---- phase 2: pyramid means, VT^T, WV2 ----
        pfx = consts.tile([P, 4, S], BF16)
        _psp_stage2(nc, poolb, g24x, 4, pfx)
        kpx = consts.tile([P, 2, SP], FP8)
        _psp_stage2(nc, poolb, g24k, 2, kpx)

        # VT^T = Wv @ PFX : [2*128 (v), 110 (s)]
        vtT = consts.tile([P, 2, S], BF16)
        for vc in range(2):
            vtT_ps = psB.tile([P, S], F32, tag="sim", name=f"vtT_ps{vc}")
            for kc in range(4):
                nc.tensor.matmul(vtT_ps, wvt[:, kc, ts(vc, P)], pfx[:, kc, :],
                                 start=(kc == 0), stop=(kc == 3))
            nc.scalar.copy(vtT[:, vc, :], vtT_ps)

        # WV2 = VT @ Wc^T : [110 (s), 512 (out)] -- stationary for the
        # attention->output matmul (folds the value->out conv into VT)
        wv2_ps = psB.tile([P, NT], F32, tag="csb", name="wv2_ps")
        for vc in range(2):
            nc.tensor.matmul(wv2_ps[:S, :], vtT[:, vc, :], wct[:, vc, :],
                             start=(vc == 0), stop=(vc == 1))
        wv2 = consts.tile([S, COUT], BF16)
        nc.scalar.copy(wv2, wv2_ps[:S, :])

        # ---- phase 3: attention + output, software-pipelined over pairs of
        # N tiles: pair k's softmax chain (ACT exp -> PE colsum -> DVE recip
        # -> GpSimd mult) fills while pair k-1's 16 output matmuls stream. ----
        def out_stage(tt, ens):
            for mc in range(4):
                ops = psO.tile([P, 2, NT], F32, tag="outp", name=f"ops{tt[0]}_{mc}")
                nc.tensor.matmul(ops[:, 0, :], ident, outb[:, mc, ts(tt[0], NT)],
                                 start=True, stop=False)
                nc.tensor.matmul(ops[:, 1, :], ident, outb[:, mc, ts(tt[1], NT)],
                                 start=True, stop=False)
                nc.tensor.matmul(ops[:, 0, :], wv2[:, ts(mc, P)], ens[0][:S, :],
                                 start=False, stop=True)
                nc.tensor.matmul(ops[:, 1, :], wv2[:, ts(mc, P)], ens[1][:S, :],
                                 start=False, stop=True)
                osb = outp.tile([P, 2, NT], BF16, tag="osb")
                opsv = ops.rearrange("p j n -> p (j n)")
                osv = osb.rearrange("p j n -> p (j n)")
                if mc < 2:
                    nc.scalar.activation(osv, opsv, RELU, bias=bfb[:, mc:mc + 1])
                else:
                    with nc.allow_low_precision(reason="output bf16"):
                        nc.vector.tensor_scalar(osv, opsv,
                                                scalar1=bfb[:, mc:mc + 1],
                                                scalar2=0.0, op0=ADD, op1=MAX)
                nc.sync.dma_start(ov[:, mc, ds(tt[0] * NT, 2 * NT)], osv)

        prev = None
        for tp in range(NTILES // 2):
            tt = (2 * tp, 2 * tp + 1)
            sims = []
            for t in tt:
                sim_ps = psB.tile([P, NT], F32, tag="sim")
                nc.tensor.matmul(sim_ps[:S, :], kpx[:, 0:2, 0:S],
                                 kfb[:, 0:2, ds(t * NT, NT)],
                                 start=True, stop=True, perf_mode=DR)
                sims.append(sim_ps)
            e1s = []
            for sim_ps in sims:
                e1 = work.tile([P, NT], BF16, tag="e1")
                nc.scalar.activation(e1[:S, :], sim_ps[:S, :], EXP, scale=0.0625)
                e1s.append(e1)
            if prev is not None:
                out_stage(*prev)
            ens = []
            for e1 in e1s:
                # colsum broadcast to all 110 partitions in one matmul
                csb_ps = psB.tile([P, NT], F32, tag="csb")
                nc.tensor.matmul(csb_ps[:S, :], ones_sxs, e1[:S, :],
                                 start=True, stop=True)
                rcol = work.tile([P, NT], F32, tag="rcol")
                nc.vector.reciprocal_approx_fast(rcol[:S, :], csb_ps[:S, :])
                en = work.tile([P, NT], BF16, tag="en")
                with nc.allow_low_precision(reason="softmax norm"):
                    nc.gpsimd.tensor_mul(en[:S, :], e1[:S, :], rcol[:S, :])
                ens.append(en)
            prev = (tt, ens)
        out_stage(*prev)


def _prune_redundant_ldweights(nc):
    """Remove back-to-back InstLdweights that reload the exact same stationary
    operand (walrus emits one LDWEIGHTS per matmul; our stationary-major loop
    order makes most of them redundant, and dropping them lets consecutive
    matmuls pipeline their fill/drain). All stationaries in this kernel are
    written exactly once before first use, so a signature match is sufficient.
    """
    n_pruned = 0
    for f in nc.m.functions:
        for blk in f.blocks:
            insts = list(blk.instructions)
            out = []
            last_sig = None
            for inst in insts:
                tname = type(inst).__name__
                eng = getattr(inst, "engine", None)
                if eng == mybir.EngineType.PE:
                    if tname == "InstLdweights":
                        ap = inst.ins[0]
                        sig = (ap.memref, ap.offset, str(ap.ap), str(ap.dtype),
                               str(inst.perf_mode), str(inst.is_transpose),
                               str(inst.tile_position), str(inst.tile_size))
                        if sig == last_sig and inst.sync_info is None:
                            n_pruned += 1
                            continue
                        last_sig = sig
                    elif tname in ("InstMatmult", "InstEventSemaphore"):
                        pass
                    else:
                        last_sig = None
                out.append(inst)
            blk.instructions = out
    return n_pruned


def _patch_act_tables():
    """Force every activation onto the one table that holds Exp, Relu and
    Copy together (`natural_log_exp_and_others`), so the kernel does a single
    ACT_TABLE_LOAD instead of reloading on table switches.

    Table ids are positional (index into act_info.json), so we keep the dict
    order/size and just empty the other entries.
    """
    import concourse.hw_specs as hw_specs

    if getattr(bacc, "_apnb_act_patch", False):
        return
    orig = hw_specs.get_activation_tables

    def patched(module_arch):
        tabs = orig(module_arch)
        keep = "natural_log_exp_and_others"
        if keep not in tabs:
            return tabs
        return {k: (v if k == keep else set()) for k, v in tabs.items()}

    bacc.get_activation_tables = patched
    bacc._apnb_act_patch = True


def build_nc():
    _patch_act_tables()
    nc = bacc.Bacc("TRN2", target_bir_lowering=False, debug=False)
    x_d = nc.declare_dram_parameter("x", [CIN, N], BF16, isOutput=False)
    x8_d = nc.declare_dram_parameter("x8", [CIN, N], FP8, isOutput=False)
    wk8_d = nc.declare_dram_parameter("wk8", [P, 4 * CK], FP8, isOutput=False)
    wvt_d = nc.declare_dram_parameter("wvt", [P, 4 * CV], BF16, isOutput=False)
    wct_d = nc.declare_dram_parameter("wct", [P, 2 * COUT], BF16, isOutput=False)
    bwt_d = nc.declare_dram_parameter("bwt", [P, 4 * COUT], BF16, isOutput=False)
    bk_d = nc.declare_dram_parameter("bk", [P, 2], F32, isOutput=False)
    bf_d = nc.declare_dram_parameter("bf", [P, 4], F32, isOutput=False)
    out_d = nc.declare_dram_parameter("out", [COUT, N], BF16, isOutput=True)
    with tile.TileContext(nc) as tc:
        with ExitStack() as ctx:
            _build_body(ctx, tc, x_d, x8_d, wk8_d, wvt_d, wct_d, bwt_d, bk_d,
                        bf_d, out_d)
    nc.compile()
    _prune_redundant_ldweights(nc)
    return nc


def _get_nc():
    global _NC
    if _NC is None:
        _NC = build_nc()
    return _NC


def fold_params(Wk, bk, gk, betak, mk, vk, Wv, bv, Ww, bw, Wo, bo, go, betao,
                mo, vo):
    """Fold BN params + the Ww conv into effective weights (all f32 numpy)."""
    bf16 = ml_dtypes.bfloat16
    fp8 = ml_dtypes.float8_e4m3
    sk = gk / np.sqrt(vk + EPS)
    Wk_f = sk[:, None] * Wk
    bk_f = (bk - mk) * sk + betak
    so = go / np.sqrt(vo + EPS)
    A = so[:, None] * Wo[:, :CIN]      # applies to ctx2 = Ww@ctx + bw
    Bw = so[:, None] * Wo[:, CIN:]     # applies to feats
    b0 = (bo - mo) * so + betao
    Wc = A @ Ww                        # (COUT, CV)
    # attn rows sum to 1  =>  value bias bv contributes Wc @ bv everywhere
    bf_ = b0 + A @ bw + Wc @ bv
    def pack(Wt):
        # [kc*128+p, m] -> [p, kc*M+m]: one contiguous DMA row per partition
        kc = Wt.shape[0] // P
        return np.ascontiguousarray(
            Wt.reshape(kc, P, -1).transpose(1, 0, 2).reshape(P, -1))

    return {
        "wk8": pack(Wk_f.T * WKSCALE).astype(fp8),
        "wvt": pack(Wv.T).astype(bf16),
        "wct": pack(Wc.T).astype(bf16),
        "bwt": pack(Bw.T).astype(bf16),
        "bk": pack(bk_f[:, None]).astype(np.float32),
        "bf": pack(bf_[:, None]).astype(np.float32),
    }


def kernel(**inputs):
    global LAST_RESULT
    feats = np.asarray(inputs["feats"], np.float32)
    B = feats.shape[0]
    assert feats.shape == (B, CIN, H, W) and B == N_CORES

    common = fold_params(
        np.asarray(inputs["Wk"], np.float32), np.asarray(inputs["bk"], np.float32),
        np.asarray(inputs["gk"], np.float32), np.asarray(inputs["betak"], np.float32),
        np.asarray(inputs["mk"], np.float32), np.asarray(inputs["vk"], np.float32),
        np.asarray(inputs["Wv"], np.float32), np.asarray(inputs["bv"], np.float32),
        np.asarray(inputs["Ww"], np.float32), np.asarray(inputs["bw"], np.float32),
        np.asarray(inputs["Wo"], np.float32), np.asarray(inputs["bo"], np.float32),
        np.asarray(inputs["go"], np.float32), np.asarray(inputs["betao"], np.float32),
        np.asarray(inputs["mo"], np.float32), np.asarray(inputs["vo"], np.float32),
    )
    bf16 = ml_dtypes.bfloat16
    fp8 = ml_dtypes.float8_e4m3
    in_maps = []
    for i in range(N_CORES):
        xi = np.ascontiguousarray(feats[i].reshape(CIN, N))
        in_maps.append({"x": xi.astype(bf16), "x8": xi.astype(fp8), **common})
    nc = _get_nc()
    res = run_bass_kernel_spmd(nc, in_maps, core_ids=list(range(N_CORES)),
                               trace=PROFILE)
    LAST_RESULT = res
    out = np.stack([res.results[i]["out"].astype(np.float32).reshape(COUT, H, W)
                    for i in range(N_CORES)])
    return out


# revision 16
# speedup vs baseline: 1.0219x; 1.0094x over previous
"""APNB (asymmetric pyramid non-local block) sparse-attention kernel for 8 TRN2 NeuronCores.

Strategy: pure data-parallel over batch (B=8 -> one batch element per core, no
collectives). Per core, the whole block is computed with TensorE GEMMs
(f32 PSUM accumulation), bf16 on the accuracy-critical output path and
fp8-e4m3 DoubleRow (2x rate) on the attention path:

  host:        BN+bias folded into conv weights; W (value->out conv) is folded
               through the pooled value matrix on-device (WV2 = (Wc @ VT^T)^T),
               so the attention contribution to the output needs only a
               110-deep contraction instead of 256-deep ctx + 512x256 conv.
               X shipped twice: bf16 (output path) and fp8 (key path, 2x PE).
               Wk pre-scaled by 64 into fp8 range; un-scaled in the KF drain.
  phase 1 (streamed over blocks of 2 N-tiles):
               KF   = relu((Wk'*64 @ X8)/64 + bk')  fp8 DoubleRow matmuls
                                                    (256, 9216) persistent fp8
               OUTB = Bw' @ X                       bf16, (512, 9216) persistent
               col-pool partial sums of X and KF    (DVE reduce)
  phase 2:     row-pool + PSP pyramid (1,3,6,8) means -> PFX (X pools, bf16),
               KPX (KF pools, fp8)
               VT^T = Wv @ PFX                      (2x128, 110)
               WV2  = VT @ Wc^T                     (110, 512) via PE
  phase 3 (streamed over pairs of N-tiles):
               SIM^T = KP^T @ KF       one fp8 DoubleRow matmul per tile
               E1    = exp(s*SIM)                   (ACT)
               CSB   = ones_SxS @ E1   colsum broadcast to 110 partitions (PE)
               ATTN  = E1 * reciprocal(CSB)         (DVE, div-free softmax)
               OUT   = relu(OUTB + WV2^T @ ATTN + b')  (OUTB re-injected into
                       PSUM with an identity matmul; drains split ACT/DVE)
               OUT written bf16 (halves output DMA), upcast to f32 on host.

The softmax needs no max-subtraction: |s*sim| is O(1) for this problem's data
distribution, so exp is safe in f32. fp8 on the attention path is safe because
the attention context contributes only a few percent of the output magnitude.
"""

import numpy as np
import ml_dtypes
from contextlib import ExitStack

import concourse.bass as bass
import concourse.bacc as bacc
import concourse.mybir as mybir
import concourse.tile as tile
from concourse.bass import ts, ds
from concourse.bass_utils import run_bass_kernel_spmd
from concourse.masks import make_identity

P = 128
CIN, CK, CV, COUT = 512, 256, 256, 512
H = W = 96
N = H * W              # 9216
NT = 512               # matmul free-dim tile
NTILES = N // NT       # 18
NB = 2                 # tiles per phase-1 block
S = 110                # pooled tokens: 1+9+36+64
SP = 112               # padded S so the fp8 DoubleRow k-group step is 16-aligned
EPS = 1e-5
WKSCALE = 64.0         # Wk prescale into fp8-normal range
F32 = mybir.dt.float32
BF16 = mybir.dt.bfloat16
FP8 = mybir.dt.float8e4
RELU = mybir.ActivationFunctionType.Relu
EXP = mybir.ActivationFunctionType.Exp
COPY = mybir.ActivationFunctionType.Copy
ADD = mybir.AluOpType.add
MAX = mybir.AluOpType.max
DR = mybir.MatmulPerfMode.DoubleRow
AX = mybir.AxisListType

N_CORES = 8

# set by test harness to capture profile info
PROFILE = False
LAST_RESULT = None

_NC = None


def _psp_stage2(nc, pool, g24, ko, pf):
    """g24: [P, ko, 24, 24] 4x4-pixel SUMS. pf: [P, ko, 110+] pyramid means.

    Reference concat order (1, 3, 6, 8). Scale-6 cells are 4x4 grid cells;
    scale-8 are 3x3; scale-3 = 2x2 of scale-6; scale-1 = sum of all scale-3.
    """
    f = F32
    lp = nc.allow_low_precision
    # ---- scale 6 (cells of 4x4 grid entries = 16x16 px) ----
    c6 = pool.tile([P, ko, 24, 6], BF16, tag="c6")
    with lp(reason="pool partials"):
        nc.vector.reduce_sum(c6, g24.rearrange("p k a (b bi) -> p k a b bi", bi=4), axis=AX.X)
    v6 = c6.rearrange("p k (a ai) b -> p k a ai b", ai=4)
    s6 = pool.tile([P, ko, 6, 6], f, tag="s6")
    nc.vector.tensor_add(s6, v6[:, :, :, 0, :], v6[:, :, :, 1, :])
    nc.vector.tensor_add(s6, s6, v6[:, :, :, 2, :])
    nc.vector.tensor_add(s6, s6, v6[:, :, :, 3, :])
    with lp(reason="pyramid means"):
        nc.vector.tensor_scalar_mul(
            pf[:, :, 10:46], s6.rearrange("p k a b -> p k (a b)"), 1.0 / 256.0
        )
    # ---- scale 3 (2x2 of scale-6 cells = 32x32 px) ----
    c3 = pool.tile([P, ko, 6, 3], f, tag="c3")
    nc.vector.reduce_sum(c3, s6.rearrange("p k a (b bi) -> p k a b bi", bi=2), axis=AX.X)
    v3 = c3.rearrange("p k (a ai) b -> p k a ai b", ai=2)
    s3 = pool.tile([P, ko, 3, 3], f, tag="s3")
    nc.vector.tensor_add(s3, v3[:, :, :, 0, :], v3[:, :, :, 1, :])
    with lp(reason="pyramid means"):
        nc.vector.tensor_scalar_mul(
            pf[:, :, 1:10], s3.rearrange("p k a b -> p k (a b)"), 1.0 / 1024.0
        )
    # ---- scale 1 ----
    t1 = pool.tile([P, ko, 1], f, tag="t1")
    nc.vector.reduce_sum(t1, s3.rearrange("p k a b -> p k (a b)"), axis=AX.X)
    with lp(reason="pyramid means"):
        nc.vector.tensor_scalar_mul(pf[:, :, 0:1], t1, 1.0 / 9216.0)
    # ---- scale 8 (cells of 3x3 grid entries = 12x12 px) ----
    c8 = pool.tile([P, ko, 24, 8], BF16, tag="c8")
    with lp(reason="pool partials"):
        nc.vector.reduce_sum(c8, g24.rearrange("p k a (b bi) -> p k a b bi", bi=3), axis=AX.X)
    v8 = c8.rearrange("p k (a ai) b -> p k a ai b", ai=3)
    s8 = pool.tile([P, ko, 8, 8], f, tag="s8")
    nc.vector.tensor_add(s8, v8[:, :, :, 0, :], v8[:, :, :, 1, :])
    nc.vector.tensor_add(s8, s8, v8[:, :, :, 2, :])
    with lp(reason="pyramid means"):
        nc.vector.tensor_scalar_mul(
            pf[:, :, 46:110], s8.rearrange("p k a b -> p k (a b)"), 1.0 / 144.0
        )


def _build_body(ctx: ExitStack, tc: tile.TileContext, x_d, x8_d, wk8_d, wvt_d,
                wct_d, bwt_d, bk_d, bf_d, out_d):
    nc = tc.nc

    consts = ctx.enter_context(tc.tile_pool(name="consts", bufs=1))
    big = ctx.enter_context(tc.tile_pool(name="big", bufs=1))
    stage = ctx.enter_context(tc.tile_pool(name="stage", bufs=2))
    poolb = ctx.enter_context(tc.tile_pool(name="poolb", bufs=1))
    work = ctx.enter_context(tc.tile_pool(name="work", bufs=4))
    outp = ctx.enter_context(tc.tile_pool(name="outp", bufs=3))

    # ---- phase-1-critical weights first so the first matmul isn't stuck
    # behind phase-2/3 constants on the DMA queue (all weight params are
    # host-packed [P, ...] so each partition is one contiguous descriptor) ----
    wk8 = consts.tile([P, 4, CK], FP8)
    nc.sync.dma_start(wk8, wk8_d[:])
    bkb = consts.tile([P, 2], F32)
    nc.sync.dma_start(bkb, bk_d[:])
    bwt = consts.tile([P, 4, COUT], BF16)

    # ---- persistent full-res activations ----
    kfb = big.tile([P, 2, N], FP8)     # relu(key/query features)
    outb = big.tile([P, 4, N], BF16)   # Bw' @ X partial of the output conv
    gcolx = poolb.tile([P, 4, 2304], BF16)  # X col-pool sums (4-px groups)
    g24k = poolb.tile([P, 2, 24, 24], F32)  # KF 4x4-block sums (24x24 grid)

    xv = x_d[:].rearrange("(kc p) n -> p kc n", p=P)
    x8v = x8_d[:].rearrange("(kc p) n -> p kc n", p=P)
    ov = out_d[:].rearrange("(mc p) n -> p mc n", p=P)
    kg = kfb.rearrange("p k (h w) -> p k h w", w=W)
    hc_done = 0

    # ---- phase 1: stream X in blocks of NB tiles; KF (fp8 DoubleRow),
    # OUTB (bf16), col-pools. Stationary-major loop order so one LDWEIGHTS
    # serves NB matmuls. ----
    nblocks = NTILES // NB
    # phase-2/3 consts, declared early so their DMAs overlap phase-1 compute
    # (issued after the first block's X DMAs below)
    wvt = consts.tile([P, 4, CV], BF16)
    wct = consts.tile([P, 2, COUT], BF16)
    bfb = consts.tile([P, 4], F32)
    g24x = poolb.tile([P, 4, 24, 24], BF16)
    gxv = gcolx.rearrange("p k (hb hi wb) -> p k hb hi wb", hi=4, wb=24)
    def kf_stage(psA, b, x8t):
        # KF: fp8 DoubleRow, contraction 512 = 2 k-pair groups
        c0 = b * NB * NT
        for mc in range(2):
            ps = psA.tile([P, NB, NT], F32, tag="kf", name=f"kps{b}_{mc}")
            for k2 in range(2):
                for j in range(NB):
                    nc.tensor.matmul(ps[:, j, :],
                                     wk8[:, 2 * k2:2 * k2 + 2, ts(mc, P)],
                                     x8t[:, 2 * k2:2 * k2 + 2, ts(j, NT)],
                                     start=(k2 == 0), stop=(k2 == 1),
                                     perf_mode=DR)
            nc.scalar.activation(kfb[:, mc, ds(c0, NB * NT)],
                                 ps.rearrange("p j n -> p (j n)"), RELU,
                                 bias=bkb[:, mc:mc + 1], scale=1.0 / WKSCALE)
        # KF pooling straight to the 24x24 grid, in 12-row chunks (kfb is
        # persistent, so chunks can span tile boundaries)
        nonlocal hc_done
        while hc_done < 8 and (hc_done + 1) * 1152 <= c0 + NB * NT:
            hc = hc_done
            for k in range(2):
                src_ap = kg[:, k, ts(hc, 12), :].rearrange(
                    "p (hb hi) (wb wi) -> p hb wb hi wi", hi=4, wi=4)
                nc.vector.reduce_sum(g24k[:, k, ts(hc, 3), :], src_ap,
                                     axis=AX.XY)
            hc_done += 1

    def outb_stage(psA, b, xt):
        # OUTB: bf16, mc 0..3
        c0 = b * NB * NT
        for mc in range(4):
            ps = psA.tile([P, NB, NT], F32, tag="ob", name=f"obs{b}_{mc}")
            for kc in range(4):
                for j in range(NB):
                    nc.tensor.matmul(ps[:, j, :], bwt[:, kc, ts(mc, P)],
                                     xt[:, kc, ts(j, NT)],
                                     start=(kc == 0), stop=(kc == 3))
            nc.scalar.activation(outb[:, mc, ds(c0, NB * NT)],
                                 ps.rearrange("p j n -> p (j n)"), COPY)

    def xpool_stage(b, xt):
        # X col-pool for this block (4-px groups along w). Stage 1 is a
        # packed pair-add on DVE (16-bit 2x mode: (x0+x2, x1+x3)); stage 2
        # adds the interleaved pair on idle GpSimd.
        ng = NB * P  # 4-px groups in this block
        xg = xt.rearrange("p k (g gi) -> p k g gi", gi=4)
        gct = stage.tile([P, 4, ng, 2], BF16, tag="gct", name=f"gct{b}")
        with nc.allow_low_precision(reason="pyramid-pool partials in bf16"):
            nc.vector.tensor_add(gct, xg[:, :, :, 0:2], xg[:, :, :, 2:4])
            nc.gpsimd.tensor_add(gcolx[:, :, ds(b * ng, ng)],
                                 gct[:, :, :, 0], gct[:, :, :, 1])
        # X row-pool onto the 24x24 grid, in 8-row-of-grid chunks
        # (8 grid rows = 32 px rows = 3072 px = 3 blocks)
        if b % 3 == 2:
            hb0 = (b // 3) * 8
            sl = (slice(None), slice(None), ds(hb0, 8))
            nc.vector.tensor_add(g24x[sl], gxv[sl + (0,)], gxv[sl + (1,)])
            nc.vector.tensor_add(g24x[sl], g24x[sl], gxv[sl + (2,)])
            nc.vector.tensor_add(g24x[sl], g24x[sl], gxv[sl + (3,)])

    # Phase-1 software pipeline: KF for block b runs one block ahead of OUTB
    # for block b-1, so the final KF-pool -> pyramid chain overlaps the last
    # OUTB matmul stream and the startup only waits for x8 (not xt+bwt).
    with tc.tile_pool(name="psA", bufs=2, space="PSUM") as psA:
        prev_xt = None
        for b in range(nblocks):
            c0 = b * NB * NT
            x8t = stage.tile([P, 4, NB * NT], FP8, tag="x8t", name=f"x8t{b}")
            for j in range(NB):
                nc.sync.dma_start(x8t[:, :, ts(j, NT)], x8v[:, :, ds(c0 + j * NT, NT)])
            if b == 0:
                nc.sync.dma_start(bwt, bwt_d[:])
            xt = stage.tile([P, 4, NB * NT], BF16, tag="xt", name=f"xt{b}")
            for j in range(NB):
                nc.sync.dma_start(xt[:, :, ts(j, NT)], xv[:, :, ds(c0 + j * NT, NT)])
            if b == 1:
                nc.sync.dma_start(wvt, wvt_d[:])
                nc.sync.dma_start(wct, wct_d[:])
                nc.sync.dma_start(bfb, bf_d[:])
            kf_stage(psA, b, x8t)
            xpool_stage(b, xt)
            if prev_xt is not None:
                outb_stage(psA, b - 1, prev_xt)
            prev_xt = xt
        outb_stage(psA, nblocks - 1, prev_xt)

    # ---- small on-chip constants ----
    ones_sxs = consts.tile([S, S], BF16)
    nc.vector.memset(ones_sxs, 1.0)
    ident = consts.tile([P, P], BF16)
    make_identity(nc, ident)

    with tc.tile_pool(name="psB", bufs=2, space="PSUM") as psB, \
         tc.tile_pool(name="psO", bufs=2, space="PSUM") as psO:
        # ---- phase 2: pyramid means, VT^T, WV2 ----
        pfx = consts.tile([P, 4, S], BF16)
        _psp_stage2(nc, poolb, g24x, 4, pfx)
        kpx = consts.tile([P, 2, SP], FP8)
        _psp_stage2(nc, poolb, g24k, 2, kpx)

        # VT^T = Wv @ PFX : [2*128 (v), 110 (s)]
        vtT = consts.tile([P, 2, S], BF16)
        for vc in range(2):
            vtT_ps = psB.tile([P, S], F32, tag="sim", name=f"vtT_ps{vc}")
            for kc in range(4):
                nc.tensor.matmul(vtT_ps, wvt[:, kc, ts(vc, P)], pfx[:, kc, :],
                                 start=(kc == 0), stop=(kc == 3))
            nc.scalar.copy(vtT[:, vc, :], vtT_ps)

        # WV2 = VT @ Wc^T : [110 (s), 512 (out)] -- stationary for the
        # attention->output matmul (folds the value->out conv into VT)
        wv2_ps = psB.tile([P, NT], F32, tag="csb", name="wv2_ps")
        for vc in range(2):
            nc.tensor.matmul(wv2_ps[:S, :], vtT[:, vc, :], wct[:, vc, :],
                             start=(vc == 0), stop=(vc == 1))
        wv2 = consts.tile([S, COUT], BF16)
        nc.scalar.copy(wv2, wv2_ps[:S, :])

        # ---- phase 3: attention + output, software-pipelined over pairs of
        # N tiles: pair k's softmax chain (ACT exp -> PE colsum -> DVE recip
        # -> GpSimd mult) fills while pair k-1's 16 output matmuls stream. ----
        def out_stage(tt, ens):
            for mc in range(4):
                ops = psO.tile([P, 2, NT], F32, tag="outp", name=f"ops{tt[0]}_{mc}")
                nc.tensor.matmul(ops[:, 0, :], ident, outb[:, mc, ts(tt[0], NT)],
                                 start=True, stop=False)
                nc.tensor.matmul(ops[:, 1, :], ident, outb[:, mc, ts(tt[1], NT)],
                                 start=True, stop=False)
                nc.tensor.matmul(ops[:, 0, :], wv2[:, ts(mc, P)], ens[0][:S, :],
                                 start=False, stop=True)
                nc.tensor.matmul(ops[:, 1, :], wv2[:, ts(mc, P)], ens[1][:S, :],
                                 start=False, stop=True)
                osb = outp.tile([P, 2, NT], BF16, tag="osb")
                opsv = ops.rearrange("p j n -> p (j n)")
                osv = osb.rearrange("p j n -> p (j n)")
                if mc >= 2:
                    nc.scalar.activation(osv, opsv, RELU, bias=bfb[:, mc:mc + 1])
                else:
                    with nc.allow_low_precision(reason="output bf16"):
                        nc.vector.tensor_scalar(osv, opsv,
                                                scalar1=bfb[:, mc:mc + 1],
                                                scalar2=0.0, op0=ADD, op1=MAX)
                nc.sync.dma_start(ov[:, mc, ds(tt[0] * NT, 2 * NT)], osv)

        prev = None
        for tp in range(NTILES // 2):
            tt = (2 * tp, 2 * tp + 1)
            sims = []
            for t in tt:
                sim_ps = psB.tile([P, NT], F32, tag="sim")
                nc.tensor.matmul(sim_ps[:S, :], kpx[:, 0:2, 0:S],
                                 kfb[:, 0:2, ds(t * NT, NT)],
                                 start=True, stop=True, perf_mode=DR)
                sims.append(sim_ps)
            e1s = []
            for sim_ps in sims:
                e1 = work.tile([P, NT], BF16, tag="e1")
                nc.scalar.activation(e1[:S, :], sim_ps[:S, :], EXP, scale=0.0625)
                e1s.append(e1)
            if prev is not None:
                out_stage(*prev)
            ens = []
            for e1 in e1s:
                # colsum broadcast to all 110 partitions in one matmul
                csb_ps = psB.tile([P, NT], F32, tag="csb")
                nc.tensor.matmul(csb_ps[:S, :], ones_sxs, e1[:S, :],
                                 start=True, stop=True)
                rcol = work.tile([P, NT], F32, tag="rcol")
                nc.vector.reciprocal_approx_fast(rcol[:S, :], csb_ps[:S, :])
                en = work.tile([P, NT], BF16, tag="en")
                with nc.allow_low_precision(reason="softmax norm"):
                    nc.gpsimd.tensor_mul(en[:S, :], e1[:S, :], rcol[:S, :])
                ens.append(en)
            prev = (tt, ens)
        out_stage(*prev)


def _prune_redundant_ldweights(nc):
    """Remove back-to-back InstLdweights that reload the exact same stationary
    operand (walrus emits one LDWEIGHTS per matmul; our stationary-major loop
    order makes most of them redundant, and dropping them lets consecutive
    matmuls pipeline their fill/drain). All stationaries in this kernel are
    written exactly once before first use, so a signature match is sufficient.
    """
    n_pruned = 0
    for f in nc.m.functions:
        for blk in f.blocks:
            insts = list(blk.instructions)
            out = []
            last_sig = None
            for inst in insts:
                tname = type(inst).__name__
                eng = getattr(inst, "engine", None)
                if eng == mybir.EngineType.PE:
                    if tname == "InstLdweights":
                        ap = inst.ins[0]
                        sig = (ap.memref, ap.offset, str(ap.ap), str(ap.dtype),
                               str(inst.perf_mode), str(inst.is_transpose),
                               str(inst.tile_position), str(inst.tile_size))
                        if sig == last_sig and inst.sync_info is None:
                            n_pruned += 1
                            continue
                        last_sig = sig
                    elif tname in ("InstMatmult", "InstEventSemaphore"):
                        pass
                    else:
                        last_sig = None
                out.append(inst)
            blk.instructions = out
    return n_pruned


def _patch_act_tables():
    """Force every activation onto the one table that holds Exp, Relu and
    Copy together (`natural_log_exp_and_others`), so the kernel does a single
    ACT_TABLE_LOAD instead of reloading on table switches.

    Table ids are positional (index into act_info.json), so we keep the dict
    order/size and just empty the other entries.
    """
    import concourse.hw_specs as hw_specs

    if getattr(bacc, "_apnb_act_patch", False):
        return
    orig = hw_specs.get_activation_tables

    def patched(module_arch):
        tabs = orig(module_arch)
        keep = "natural_log_exp_and_others"
        if keep not in tabs:
            return tabs
        return {k: (v if k == keep else set()) for k, v in tabs.items()}

    bacc.get_activation_tables = patched
    bacc._apnb_act_patch = True


def build_nc():
    _patch_act_tables()
    nc = bacc.Bacc("TRN2", target_bir_lowering=False, debug=False)
    x_d = nc.declare_dram_parameter("x", [CIN, N], BF16, isOutput=False)
    x8_d = nc.declare_dram_parameter("x8", [CIN, N], FP8, isOutput=False)
    wk8_d = nc.declare_dram_parameter("wk8", [P, 4 * CK], FP8, isOutput=False)
    wvt_d = nc.declare_dram_parameter("wvt", [P, 4 * CV], BF16, isOutput=False)
    wct_d = nc.declare_dram_parameter("wct", [P, 2 * COUT], BF16, isOutput=False)
    bwt_d = nc.declare_dram_parameter("bwt", [P, 4 * COUT], BF16, isOutput=False)
    bk_d = nc.declare_dram_parameter("bk", [P, 2], F32, isOutput=False)
    bf_d = nc.declare_dram_parameter("bf", [P, 4], F32, isOutput=False)
    out_d = nc.declare_dram_parameter("out", [COUT, N], BF16, isOutput=True)
    with tile.TileContext(nc) as tc:
        with ExitStack() as ctx:
            _build_body(ctx, tc, x_d, x8_d, wk8_d, wvt_d, wct_d, bwt_d, bk_d,
                        bf_d, out_d)
    nc.compile()
    _prune_redundant_ldweights(nc)
    return nc


def _get_nc():
    global _NC
    if _NC is None:
        _NC = build_nc()
    return _NC


def fold_params(Wk, bk, gk, betak, mk, vk, Wv, bv, Ww, bw, Wo, bo, go, betao,
                mo, vo):
    """Fold BN params + the Ww conv into effective weights (all f32 numpy)."""
    bf16 = ml_dtypes.bfloat16
    fp8 = ml_dtypes.float8_e4m3
    sk = gk / np.sqrt(vk + EPS)
    Wk_f = sk[:, None] * Wk
    bk_f = (bk - mk) * sk + betak
    so = go / np.sqrt(vo + EPS)
    A = so[:, None] * Wo[:, :CIN]      # applies to ctx2 = Ww@ctx + bw
    Bw = so[:, None] * Wo[:, CIN:]     # applies to feats
    b0 = (bo - mo) * so + betao
    Wc = A @ Ww                        # (COUT, CV)
    # attn rows sum to 1  =>  value bias bv contributes Wc @ bv everywhere
    bf_ = b0 + A @ bw + Wc @ bv
    def pack(Wt):
        # [kc*128+p, m] -> [p, kc*M+m]: one contiguous DMA row per partition
        kc = Wt.shape[0] // P
        return np.ascontiguousarray(
            Wt.reshape(kc, P, -1).transpose(1, 0, 2).reshape(P, -1))

    return {
        "wk8": pack(Wk_f.T * WKSCALE).astype(fp8),
        "wvt": pack(Wv.T).astype(bf16),
        "wct": pack(Wc.T).astype(bf16),
        "bwt": pack(Bw.T).astype(bf16),
        "bk": pack(bk_f[:, None]).astype(np.float32),
        "bf": pack(bf_[:, None]).astype(np.float32),
    }


def kernel(**inputs):
    global LAST_RESULT
    feats = np.asarray(inputs["feats"], np.float32)
    B = feats.shape[0]
    assert feats.shape == (B, CIN, H, W) and B == N_CORES

    common = fold_params(
        np.asarray(inputs["Wk"], np.float32), np.asarray(inputs["bk"], np.float32),
        np.asarray(inputs["gk"], np.float32), np.asarray(inputs["betak"], np.float32),
        np.asarray(inputs["mk"], np.float32), np.asarray(inputs["vk"], np.float32),
        np.asarray(inputs["Wv"], np.float32), np.asarray(inputs["bv"], np.float32),
        np.asarray(inputs["Ww"], np.float32), np.asarray(inputs["bw"], np.float32),
        np.asarray(inputs["Wo"], np.float32), np.asarray(inputs["bo"], np.float32),
        np.asarray(inputs["go"], np.float32), np.asarray(inputs["betao"], np.float32),
        np.asarray(inputs["mo"], np.float32), np.asarray(inputs["vo"], np.float32),
    )
    bf16 = ml_dtypes.bfloat16
    fp8 = ml_dtypes.float8_e4m3
    in_maps = []
    for i in range(N_CORES):
        xi = np.ascontiguousarray(feats[i].reshape(CIN, N))
        in_maps.append({"x": xi.astype(bf16), "x8": xi.astype(fp8), **common})
    nc = _get_nc()
    res = run_bass_kernel_spmd(nc, in_maps, core_ids=list(range(N_CORES)),
                               trace=PROFILE)
    LAST_RESULT = res
    out = np.stack([res.results[i]["out"].astype(np.float32).reshape(COUT, H, W)
                    for i in range(N_CORES)])
    return out


# revision 19
# speedup vs baseline: 1.0507x; 1.0282x over previous
"""APNB (asymmetric pyramid non-local block) sparse-attention kernel for 8 TRN2 NeuronCores.

Strategy: pure data-parallel over batch (B=8 -> one batch element per core, no
collectives). Per core, the whole block is computed with TensorE GEMMs
(f32 PSUM accumulation), bf16 on the accuracy-critical output path and
fp8-e4m3 DoubleRow (2x rate) on the attention path:

  host:        BN+bias folded into conv weights; W (value->out conv) is folded
               through the pooled value matrix on-device (WV2 = (Wc @ VT^T)^T),
               so the attention contribution to the output needs only a
               110-deep contraction instead of 256-deep ctx + 512x256 conv.
               X shipped twice: bf16 (output path) and fp8 (key path, 2x PE).
               Wk pre-scaled by 64 into fp8 range; un-scaled in the KF drain.
  phase 1 (streamed over blocks of 2 N-tiles):
               KF   = relu((Wk'*64 @ X8)/64 + bk')  fp8 DoubleRow matmuls
                                                    (256, 9216) persistent fp8
               OUTB = Bw' @ X                       bf16, (512, 9216) persistent
               col-pool partial sums of X and KF    (DVE reduce)
  phase 2:     row-pool + PSP pyramid (1,3,6,8) means -> PFX (X pools, bf16),
               KPX (KF pools, fp8)
               VT^T = Wv @ PFX                      (2x128, 110)
               WV2  = VT @ Wc^T                     (110, 512) via PE
  phase 3 (streamed over pairs of N-tiles):
               SIM^T = KP^T @ KF       one fp8 DoubleRow matmul per tile
               E1    = exp(s*SIM)                   (ACT)
               CSB   = ones_SxS @ E1   colsum broadcast to 110 partitions (PE)
               ATTN  = E1 * reciprocal(CSB)         (DVE, div-free softmax)
               OUT   = relu(OUTB + WV2^T @ ATTN + b')  (OUTB re-injected into
                       PSUM with an identity matmul; drains split ACT/DVE)
               OUT written bf16 (halves output DMA), upcast to f32 on host.

The softmax needs no max-subtraction: |s*sim| is O(1) for this problem's data
distribution, so exp is safe in f32. fp8 on the attention path is safe because
the attention context contributes only a few percent of the output magnitude.
"""

import numpy as np
import ml_dtypes
from contextlib import ExitStack

import concourse.bass as bass
import concourse.bacc as bacc
import concourse.mybir as mybir
import concourse.tile as tile
from concourse.bass import ts, ds
from concourse.bass_utils import run_bass_kernel_spmd
from concourse.masks import make_identity

P = 128
CIN, CK, CV, COUT = 512, 256, 256, 512
H = W = 96
N = H * W              # 9216
NT = 512               # matmul free-dim tile
NTILES = N // NT       # 18
NB = 2                 # tiles per phase-1 block
S = 110                # pooled tokens: 1+9+36+64
SP = 112               # padded S so the fp8 DoubleRow k-group step is 16-aligned
EPS = 1e-5
WKSCALE = 64.0         # Wk prescale into fp8-normal range
F32 = mybir.dt.float32
BF16 = mybir.dt.bfloat16
FP8 = mybir.dt.float8e4
RELU = mybir.ActivationFunctionType.Relu
EXP = mybir.ActivationFunctionType.Exp
COPY = mybir.ActivationFunctionType.Copy
ADD = mybir.AluOpType.add
MAX = mybir.AluOpType.max
DR = mybir.MatmulPerfMode.DoubleRow
AX = mybir.AxisListType

N_CORES = 8

# set by test harness to capture profile info
PROFILE = False
LAST_RESULT = None

_NC = None


def _psp_stage2(nc, pool, g24, ko, pf):
    """g24: [P, ko, 24, 24] 4x4-pixel SUMS. pf: [P, ko, 110+] pyramid means.

    Reference concat order (1, 3, 6, 8). Scale-6 cells are 4x4 grid cells;
    scale-8 are 3x3; scale-3 = 2x2 of scale-6; scale-1 = sum of all scale-3.
    """
    f = F32
    lp = nc.allow_low_precision
    # ---- scale 6 (cells of 4x4 grid entries = 16x16 px) ----
    c6 = pool.tile([P, ko, 24, 6], BF16, tag="c6")
    with lp(reason="pool partials"):
        nc.vector.reduce_sum(c6, g24.rearrange("p k a (b bi) -> p k a b bi", bi=4), axis=AX.X)
    v6 = c6.rearrange("p k (a ai) b -> p k a ai b", ai=4)
    s6 = pool.tile([P, ko, 6, 6], f, tag="s6")
    nc.vector.tensor_add(s6, v6[:, :, :, 0, :], v6[:, :, :, 1, :])
    nc.vector.tensor_add(s6, s6, v6[:, :, :, 2, :])
    nc.vector.tensor_add(s6, s6, v6[:, :, :, 3, :])
    with lp(reason="pyramid means"):
        nc.vector.tensor_scalar_mul(
            pf[:, :, 10:46], s6.rearrange("p k a b -> p k (a b)"), 1.0 / 256.0
        )
    # ---- scale 3 (2x2 of scale-6 cells = 32x32 px) ----
    c3 = pool.tile([P, ko, 6, 3], f, tag="c3")
    nc.vector.reduce_sum(c3, s6.rearrange("p k a (b bi) -> p k a b bi", bi=2), axis=AX.X)
    v3 = c3.rearrange("p k (a ai) b -> p k a ai b", ai=2)
    s3 = pool.tile([P, ko, 3, 3], f, tag="s3")
    nc.vector.tensor_add(s3, v3[:, :, :, 0, :], v3[:, :, :, 1, :])
    with lp(reason="pyramid means"):
        nc.vector.tensor_scalar_mul(
            pf[:, :, 1:10], s3.rearrange("p k a b -> p k (a b)"), 1.0 / 1024.0
        )
    # ---- scale 1 ----
    t1 = pool.tile([P, ko, 1], f, tag="t1")
    nc.vector.reduce_sum(t1, s3.rearrange("p k a b -> p k (a b)"), axis=AX.X)
    with lp(reason="pyramid means"):
        nc.vector.tensor_scalar_mul(pf[:, :, 0:1], t1, 1.0 / 9216.0)
    # ---- scale 8 (cells of 3x3 grid entries = 12x12 px) ----
    c8 = pool.tile([P, ko, 24, 8], BF16, tag="c8")
    with lp(reason="pool partials"):
        nc.vector.reduce_sum(c8, g24.rearrange("p k a (b bi) -> p k a b bi", bi=3), axis=AX.X)
    v8 = c8.rearrange("p k (a ai) b -> p k a ai b", ai=3)
    s8 = pool.tile([P, ko, 8, 8], f, tag="s8")
    nc.vector.tensor_add(s8, v8[:, :, :, 0, :], v8[:, :, :, 1, :])
    nc.vector.tensor_add(s8, s8, v8[:, :, :, 2, :])
    with lp(reason="pyramid means"):
        nc.vector.tensor_scalar_mul(
            pf[:, :, 46:110], s8.rearrange("p k a b -> p k (a b)"), 1.0 / 144.0
        )


def _build_body(ctx: ExitStack, tc: tile.TileContext, x_d, x8_d, wk8_d, wvt_d,
                wct_d, bwt_d, bk_d, bf_d, out_d):
    nc = tc.nc

    consts = ctx.enter_context(tc.tile_pool(name="consts", bufs=1))
    big = ctx.enter_context(tc.tile_pool(name="big", bufs=1))
    stage = ctx.enter_context(tc.tile_pool(name="stage", bufs=2))
    poolb = ctx.enter_context(tc.tile_pool(name="poolb", bufs=1))
    work = ctx.enter_context(tc.tile_pool(name="work", bufs=6))
    outp = ctx.enter_context(tc.tile_pool(name="outp", bufs=3))

    # ---- phase-1-critical weights first so the first matmul isn't stuck
    # behind phase-2/3 constants on the DMA queue (all weight params are
    # host-packed [P, ...] so each partition is one contiguous descriptor) ----
    wk8 = consts.tile([P, 4, CK], FP8)
    nc.sync.dma_start(wk8, wk8_d[:])
    bkb = consts.tile([P, 2], F32)
    nc.sync.dma_start(bkb, bk_d[:])
    bwt = consts.tile([P, 4, COUT], BF16)

    # ---- persistent full-res activations ----
    kfb = big.tile([P, 2, N], FP8)     # relu(key/query features)
    outb = big.tile([P, 4, N], BF16)   # Bw' @ X partial of the output conv
    gcolx = poolb.tile([P, 4, 2304], BF16)  # X col-pool sums (4-px groups)
    g24k = poolb.tile([P, 2, 24, 24], F32)  # KF 4x4-block sums (24x24 grid)

    xv = x_d[:].rearrange("(kc p) n -> p kc n", p=P)
    x8v = x8_d[:].rearrange("(kc p) n -> p kc n", p=P)
    ov = out_d[:].rearrange("(mc p) n -> p mc n", p=P)
    kg = kfb.rearrange("p k (h w) -> p k h w", w=W)
    hc_done = 0

    # ---- phase 1: stream X in blocks of NB tiles; KF (fp8 DoubleRow),
    # OUTB (bf16), col-pools. Stationary-major loop order so one LDWEIGHTS
    # serves NB matmuls. ----
    nblocks = NTILES // NB
    # phase-2/3 consts, declared early so their DMAs overlap phase-1 compute
    # (issued after the first block's X DMAs below)
    wvt = consts.tile([P, 4, CV], BF16)
    wct = consts.tile([P, 2, COUT], BF16)
    bfb = consts.tile([P, 4], F32)
    g24x = poolb.tile([P, 4, 24, 24], BF16)
    gxv = gcolx.rearrange("p k (hb hi wb) -> p k hb hi wb", hi=4, wb=24)
    def kf_stage(psA, b, x8t):
        # KF: fp8 DoubleRow, contraction 512 = 2 k-pair groups
        c0 = b * NB * NT
        for mc in range(2):
            ps = psA.tile([P, NB, NT], F32, tag="kf", name=f"kps{b}_{mc}")
            for k2 in range(2):
                for j in range(NB):
                    nc.tensor.matmul(ps[:, j, :],
                                     wk8[:, 2 * k2:2 * k2 + 2, ts(mc, P)],
                                     x8t[:, 2 * k2:2 * k2 + 2, ts(j, NT)],
                                     start=(k2 == 0), stop=(k2 == 1),
                                     perf_mode=DR)
            nc.scalar.activation(kfb[:, mc, ds(c0, NB * NT)],
                                 ps.rearrange("p j n -> p (j n)"), RELU,
                                 bias=bkb[:, mc:mc + 1], scale=1.0 / WKSCALE)
        # KF pooling straight to the 24x24 grid, in 12-row chunks (kfb is
        # persistent, so chunks can span tile boundaries)
        nonlocal hc_done
        while hc_done < 8 and (hc_done + 1) * 1152 <= c0 + NB * NT:
            hc = hc_done
            for k in range(2):
                src_ap = kg[:, k, ts(hc, 12), :].rearrange(
                    "p (hb hi) (wb wi) -> p hb wb hi wi", hi=4, wi=4)
                nc.vector.reduce_sum(g24k[:, k, ts(hc, 3), :], src_ap,
                                     axis=AX.XY)
            hc_done += 1

    def outb_stage(psA, b, xt):
        # OUTB: bf16, mc 0..3
        c0 = b * NB * NT
        for mc in range(4):
            ps = psA.tile([P, NB, NT], F32, tag="ob", name=f"obs{b}_{mc}")
            for kc in range(4):
                for j in range(NB):
                    nc.tensor.matmul(ps[:, j, :], bwt[:, kc, ts(mc, P)],
                                     xt[:, kc, ts(j, NT)],
                                     start=(kc == 0), stop=(kc == 3))
            nc.scalar.activation(outb[:, mc, ds(c0, NB * NT)],
                                 ps.rearrange("p j n -> p (j n)"), COPY)

    def xpool_stage(b, xt):
        # X col-pool for this block (4-px groups along w). Stage 1 is a
        # packed pair-add on DVE (16-bit 2x mode: (x0+x2, x1+x3)); stage 2
        # adds the interleaved pair on idle GpSimd.
        ng = NB * P  # 4-px groups in this block
        xg = xt.rearrange("p k (g gi) -> p k g gi", gi=4)
        gct = stage.tile([P, 4, ng, 2], BF16, tag="gct", name=f"gct{b}")
        with nc.allow_low_precision(reason="pyramid-pool partials in bf16"):
            nc.vector.tensor_add(gct, xg[:, :, :, 0:2], xg[:, :, :, 2:4])
            nc.gpsimd.tensor_add(gcolx[:, :, ds(b * ng, ng)],
                                 gct[:, :, :, 0], gct[:, :, :, 1])
        # X row-pool onto the 24x24 grid, in 8-row-of-grid chunks
        # (8 grid rows = 32 px rows = 3072 px = 3 blocks)
        if b % 3 == 2:
            hb0 = (b // 3) * 8
            sl = (slice(None), slice(None), ds(hb0, 8))
            nc.vector.tensor_add(g24x[sl], gxv[sl + (0,)], gxv[sl + (1,)])
            nc.vector.tensor_add(g24x[sl], g24x[sl], gxv[sl + (2,)])
            nc.vector.tensor_add(g24x[sl], g24x[sl], gxv[sl + (3,)])

    with tc.tile_pool(name="psA", bufs=2, space="PSUM") as psA:
        for b in range(nblocks):
            c0 = b * NB * NT
            x8t = stage.tile([P, 4, NB * NT], FP8, tag="x8t", name=f"x8t{b}")
            for j in range(NB):
                nc.sync.dma_start(x8t[:, :, ts(j, NT)], x8v[:, :, ds(c0 + j * NT, NT)])
            if b == 0:
                nc.sync.dma_start(bwt, bwt_d[:])
            xt = stage.tile([P, 4, NB * NT], BF16, tag="xt", name=f"xt{b}")
            for j in range(NB):
                nc.sync.dma_start(xt[:, :, ts(j, NT)], xv[:, :, ds(c0 + j * NT, NT)])
            if b == 1:
                nc.sync.dma_start(wvt, wvt_d[:])
                nc.sync.dma_start(wct, wct_d[:])
                nc.sync.dma_start(bfb, bf_d[:])
            kf_stage(psA, b, x8t)
            outb_stage(psA, b, xt)
            xpool_stage(b, xt)

    # ---- small on-chip constants ----
    ones_sxs = consts.tile([S, S], BF16)
    nc.vector.memset(ones_sxs, 1.0)
    ident = consts.tile([P, P], BF16)
    make_identity(nc, ident)

    with tc.tile_pool(name="psB", bufs=2, space="PSUM") as psB, \
         tc.tile_pool(name="psO", bufs=2, space="PSUM") as psO:
        # ---- phase 2: pyramid means, VT^T, WV2 ----
        pfx = consts.tile([P, 4, S], BF16)
        _psp_stage2(nc, poolb, g24x, 4, pfx)
        kpx = consts.tile([P, 2, SP], FP8)
        _psp_stage2(nc, poolb, g24k, 2, kpx)

        # VT^T = Wv @ PFX : [2*128 (v), 110 (s)]
        vtT = consts.tile([P, 2, S], BF16)
        for vc in range(2):
            vtT_ps = psB.tile([P, S], F32, tag="sim", name=f"vtT_ps{vc}")
            for kc in range(4):
                nc.tensor.matmul(vtT_ps, wvt[:, kc, ts(vc, P)], pfx[:, kc, :],
                                 start=(kc == 0), stop=(kc == 3))
            nc.scalar.copy(vtT[:, vc, :], vtT_ps)

        # WV2 = VT @ Wc^T : [110 (s), 512 (out)] -- stationary for the
        # attention->output matmul (folds the value->out conv into VT)
        wv2_ps = psB.tile([P, NT], F32, tag="csb", name="wv2_ps")
        for vc in range(2):
            nc.tensor.matmul(wv2_ps[:S, :], vtT[:, vc, :], wct[:, vc, :],
                             start=(vc == 0), stop=(vc == 1))
        wv2 = consts.tile([S, COUT], BF16)
        nc.scalar.copy(wv2, wv2_ps[:S, :])

        # ---- phase 3: attention + output, software-pipelined over pairs of
        # N tiles: pair k's softmax chain (ACT exp -> PE colsum -> DVE recip
        # -> GpSimd mult) fills while pair k-1's 16 output matmuls stream. ----
        def out_stage(tt, ens):
            for mc in range(4):
                ops = psO.tile([P, 2, NT], F32, tag="outp", name=f"ops{tt[0]}_{mc}")
                nc.tensor.matmul(ops[:, 0, :], ident, outb[:, mc, ts(tt[0], NT)],
                                 start=True, stop=False)
                nc.tensor.matmul(ops[:, 1, :], ident, outb[:, mc, ts(tt[1], NT)],
                                 start=True, stop=False)
                nc.tensor.matmul(ops[:, 0, :], wv2[:, ts(mc, P)], ens[0][:S, :],
                                 start=False, stop=True)
                nc.tensor.matmul(ops[:, 1, :], wv2[:, ts(mc, P)], ens[1][:S, :],
                                 start=False, stop=True)
                osb = outp.tile([P, 2, NT], BF16, tag="osb")
                opsv = ops.rearrange("p j n -> p (j n)")
                osv = osb.rearrange("p j n -> p (j n)")
                if mc >= 2:
                    nc.scalar.activation(osv, opsv, RELU, bias=bfb[:, mc:mc + 1])
                else:
                    with nc.allow_low_precision(reason="output bf16"):
                        nc.vector.tensor_scalar(osv, opsv,
                                                scalar1=bfb[:, mc:mc + 1],
                                                scalar2=0.0, op0=ADD, op1=MAX)
                nc.sync.dma_start(ov[:, mc, ds(tt[0] * NT, 2 * NT)], osv)

        pending = []
        for tp in range(NTILES // 2):
            tt = (2 * tp, 2 * tp + 1)
            sims = []
            for t in tt:
                sim_ps = psB.tile([P, NT], F32, tag="sim")
                nc.tensor.matmul(sim_ps[:S, :], kpx[:, 0:2, 0:S],
                                 kfb[:, 0:2, ds(t * NT, NT)],
                                 start=True, stop=True, perf_mode=DR)
                sims.append(sim_ps)
            e1s = []
            for sim_ps in sims:
                e1 = work.tile([P, NT], BF16, tag="e1")
                nc.scalar.activation(e1[:S, :], sim_ps[:S, :], EXP, scale=0.0625)
                e1s.append(e1)
            if len(pending) >= 2:
                out_stage(*pending.pop(0))
            ens = []
            for e1 in e1s:
                # colsum broadcast to all 110 partitions in one matmul
                csb_ps = psB.tile([P, NT], F32, tag="csb")
                nc.tensor.matmul(csb_ps[:S, :], ones_sxs, e1[:S, :],
                                 start=True, stop=True)
                rcol = work.tile([P, NT], F32, tag="rcol")
                nc.vector.reciprocal_approx_fast(rcol[:S, :], csb_ps[:S, :])
                en = work.tile([P, NT], BF16, tag="en")
                with nc.allow_low_precision(reason="softmax norm"):
                    nc.gpsimd.tensor_mul(en[:S, :], e1[:S, :], rcol[:S, :])
                ens.append(en)
            pending.append((tt, ens))
        for item in pending:
            out_stage(*item)


def _prune_redundant_ldweights(nc):
    """Remove back-to-back InstLdweights that reload the exact same stationary
    operand (walrus emits one LDWEIGHTS per matmul; our stationary-major loop
    order makes most of them redundant, and dropping them lets consecutive
    matmuls pipeline their fill/drain). All stationaries in this kernel are
    written exactly once before first use, so a signature match is sufficient.
    """
    n_pruned = 0
    for f in nc.m.functions:
        for blk in f.blocks:
            insts = list(blk.instructions)
            out = []
            last_sig = None
            for inst in insts:
                tname = type(inst).__name__
                eng = getattr(inst, "engine", None)
                if eng == mybir.EngineType.PE:
                    if tname == "InstLdweights":
                        ap = inst.ins[0]
                        sig = (ap.memref, ap.offset, str(ap.ap), str(ap.dtype),
                               str(inst.perf_mode), str(inst.is_transpose),
                               str(inst.tile_position), str(inst.tile_size))
                        if sig == last_sig and inst.sync_info is None:
                            n_pruned += 1
                            continue
                        last_sig = sig
                    elif tname in ("InstMatmult", "InstEventSemaphore"):
                        pass
                    else:
                        last_sig = None
                out.append(inst)
            blk.instructions = out
    return n_pruned


def _patch_act_tables():
    """Force every activation onto the one table that holds Exp, Relu and
    Copy together (`natural_log_exp_and_others`), so the kernel does a single
    ACT_TABLE_LOAD instead of reloading on table switches.

    Table ids are positional (index into act_info.json), so we keep the dict
    order/size and just empty the other entries.
    """
    import concourse.hw_specs as hw_specs

    if getattr(bacc, "_apnb_act_patch", False):
        return
    orig = hw_specs.get_activation_tables

    def patched(module_arch):
        tabs = orig(module_arch)
        keep = "natural_log_exp_and_others"
        if keep not in tabs:
            return tabs
        return {k: (v if k == keep else set()) for k, v in tabs.items()}

    bacc.get_activation_tables = patched
    bacc._apnb_act_patch = True


def build_nc():
    _patch_act_tables()
    nc = bacc.Bacc("TRN2", target_bir_lowering=False, debug=False)
    x_d = nc.declare_dram_parameter("x", [CIN, N], BF16, isOutput=False)
    x8_d = nc.declare_dram_parameter("x8", [CIN, N], FP8, isOutput=False)
    wk8_d = nc.declare_dram_parameter("wk8", [P, 4 * CK], FP8, isOutput=False)
    wvt_d = nc.declare_dram_parameter("wvt", [P, 4 * CV], BF16, isOutput=False)
    wct_d = nc.declare_dram_parameter("wct", [P, 2 * COUT], BF16, isOutput=False)
    bwt_d = nc.declare_dram_parameter("bwt", [P, 4 * COUT], BF16, isOutput=False)
    bk_d = nc.declare_dram_parameter("bk", [P, 2], F32, isOutput=False)
    bf_d = nc.declare_dram_parameter("bf", [P, 4], F32, isOutput=False)
    out_d = nc.declare_dram_parameter("out", [COUT, N], BF16, isOutput=True)
    with tile.TileContext(nc) as tc:
        with ExitStack() as ctx:
            _build_body(ctx, tc, x_d, x8_d, wk8_d, wvt_d, wct_d, bwt_d, bk_d,
                        bf_d, out_d)
    nc.compile()
    _prune_redundant_ldweights(nc)
    return nc


def _get_nc():
    global _NC
    if _NC is None:
        _NC = build_nc()
    return _NC


def fold_params(Wk, bk, gk, betak, mk, vk, Wv, bv, Ww, bw, Wo, bo, go, betao,
                mo, vo):
    """Fold BN params + the Ww conv into effective weights (all f32 numpy)."""
    bf16 = ml_dtypes.bfloat16
    fp8 = ml_dtypes.float8_e4m3
    sk = gk / np.sqrt(vk + EPS)
    Wk_f = sk[:, None] * Wk
    bk_f = (bk - mk) * sk + betak
    so = go / np.sqrt(vo + EPS)
    A = so[:, None] * Wo[:, :CIN]      # applies to ctx2 = Ww@ctx + bw
    Bw = so[:, None] * Wo[:, CIN:]     # applies to feats
    b0 = (bo - mo) * so + betao
    Wc = A @ Ww                        # (COUT, CV)
    # attn rows sum to 1  =>  value bias bv contributes Wc @ bv everywhere
    bf_ = b0 + A @ bw + Wc @ bv
    def pack(Wt):
        # [kc*128+p, m] -> [p, kc*M+m]: one contiguous DMA row per partition
        kc = Wt.shape[0] // P
        return np.ascontiguousarray(
            Wt.reshape(kc, P, -1).transpose(1, 0, 2).reshape(P, -1))

    return {
        "wk8": pack(Wk_f.T * WKSCALE).astype(fp8),
        "wvt": pack(Wv.T).astype(bf16),
        "wct": pack(Wc.T).astype(bf16),
        "bwt": pack(Bw.T).astype(bf16),
        "bk": pack(bk_f[:, None]).astype(np.float32),
        "bf": pack(bf_[:, None]).astype(np.float32),
    }


def kernel(**inputs):
    global LAST_RESULT
    feats = np.asarray(inputs["feats"], np.float32)
    B = feats.shape[0]
    assert feats.shape == (B, CIN, H, W) and B == N_CORES

    common = fold_params(
        np.asarray(inputs["Wk"], np.float32), np.asarray(inputs["bk"], np.float32),
        np.asarray(inputs["gk"], np.float32), np.asarray(inputs["betak"], np.float32),
        np.asarray(inputs["mk"], np.float32), np.asarray(inputs["vk"], np.float32),
        np.asarray(inputs["Wv"], np.float32), np.asarray(inputs["bv"], np.float32),
        np.asarray(inputs["Ww"], np.float32), np.asarray(inputs["bw"], np.float32),
        np.asarray(inputs["Wo"], np.float32), np.asarray(inputs["bo"], np.float32),
        np.asarray(inputs["go"], np.float32), np.asarray(inputs["betao"], np.float32),
        np.asarray(inputs["mo"], np.float32), np.asarray(inputs["vo"], np.float32),
    )
    bf16 = ml_dtypes.bfloat16
    fp8 = ml_dtypes.float8_e4m3
    in_maps = []
    for i in range(N_CORES):
        xi = np.ascontiguousarray(feats[i].reshape(CIN, N))
        in_maps.append({"x": xi.astype(bf16), "x8": xi.astype(fp8), **common})
    nc = _get_nc()
    res = run_bass_kernel_spmd(nc, in_maps, core_ids=list(range(N_CORES)),
                               trace=PROFILE)
    LAST_RESULT = res
    out = np.stack([res.results[i]["out"].astype(np.float32).reshape(COUT, H, W)
                    for i in range(N_CORES)])
    return out


# revision 22
# speedup vs baseline: 1.0788x; 1.0268x over previous
"""APNB (asymmetric pyramid non-local block) sparse-attention kernel for 8 TRN2 NeuronCores.

Strategy: pure data-parallel over batch (B=8 -> one batch element per core, no
collectives). Per core, the whole block is computed with TensorE GEMMs
(f32 PSUM accumulation), bf16 on the accuracy-critical output path and
fp8-e4m3 DoubleRow (2x rate) on the attention path:

  host:        BN+bias folded into conv weights; W (value->out conv) is folded
               through the pooled value matrix on-device (WV2 = (Wc @ VT^T)^T),
               so the attention contribution to the output needs only a
               110-deep contraction instead of 256-deep ctx + 512x256 conv.
               X shipped twice: bf16 (output path) and fp8 (key path, 2x PE).
               Wk pre-scaled by 64 into fp8 range; un-scaled in the KF drain.
  phase 1 (streamed over blocks of 2 N-tiles):
               KF   = relu((Wk'*64 @ X8)/64 + bk')  fp8 DoubleRow matmuls
                                                    (256, 9216) persistent fp8
               OUTB = Bw' @ X                       bf16, (512, 9216) persistent
               col-pool partial sums of X and KF    (DVE reduce)
  phase 2:     row-pool + PSP pyramid (1,3,6,8) means -> PFX (X pools, bf16),
               KPX (KF pools, fp8)
               VT^T = Wv @ PFX                      (2x128, 110)
               WV2  = VT @ Wc^T                     (110, 512) via PE
  phase 3 (streamed over pairs of N-tiles):
               SIM^T = KP^T @ KF       one fp8 DoubleRow matmul per tile
               E1    = exp(s*SIM)                   (ACT)
               CSB   = ones_SxS @ E1   colsum broadcast to 110 partitions (PE)
               ATTN  = E1 * reciprocal(CSB)         (DVE, div-free softmax)
               OUT   = relu(OUTB + WV2^T @ ATTN + b')  (OUTB re-injected into
                       PSUM with an identity matmul; drains split ACT/DVE)
               OUT written bf16 (halves output DMA), upcast to f32 on host.

The softmax needs no max-subtraction: |s*sim| is O(1) for this problem's data
distribution, so exp is safe in f32. fp8 on the attention path is safe because
the attention context contributes only a few percent of the output magnitude.
"""

import numpy as np
import ml_dtypes
from contextlib import ExitStack

import concourse.bass as bass
import concourse.bacc as bacc
import concourse.mybir as mybir
import concourse.tile as tile
from concourse.bass import ts, ds
from concourse.bass_utils import run_bass_kernel_spmd
from concourse.masks import make_identity

P = 128
CIN, CK, CV, COUT = 512, 256, 256, 512
H = W = 96
N = H * W              # 9216
NT = 512               # matmul free-dim tile
NTILES = N // NT       # 18
NB = 2                 # tiles per phase-1 block
S = 110                # pooled tokens: 1+9+36+64
SP = 112               # padded S so the fp8 DoubleRow k-group step is 16-aligned
EPS = 1e-5
WKSCALE = 64.0         # Wk prescale into fp8-normal range
F32 = mybir.dt.float32
BF16 = mybir.dt.bfloat16
FP8 = mybir.dt.float8e4
RELU = mybir.ActivationFunctionType.Relu
EXP = mybir.ActivationFunctionType.Exp
COPY = mybir.ActivationFunctionType.Copy
ADD = mybir.AluOpType.add
MAX = mybir.AluOpType.max
DR = mybir.MatmulPerfMode.DoubleRow
AX = mybir.AxisListType

N_CORES = 8

# set by test harness to capture profile info
PROFILE = False
LAST_RESULT = None

_NC = None


def _psp_stage2(nc, pool, g24, ko, pf):
    """g24: [P, ko, 24, 24] 4x4-pixel SUMS. pf: [P, ko, 110+] pyramid means.

    Reference concat order (1, 3, 6, 8). Scale-6 cells are 4x4 grid cells;
    scale-8 are 3x3; scale-3 = 2x2 of scale-6; scale-1 = sum of all scale-3.
    """
    f = F32
    lp = nc.allow_low_precision
    # ---- scale 6 (cells of 4x4 grid entries = 16x16 px) ----
    c6 = pool.tile([P, ko, 24, 6], BF16, tag="c6")
    with lp(reason="pool partials"):
        nc.vector.reduce_sum(c6, g24.rearrange("p k a (b bi) -> p k a b bi", bi=4), axis=AX.X)
    v6 = c6.rearrange("p k (a ai) b -> p k a ai b", ai=4)
    s6 = pool.tile([P, ko, 6, 6], f, tag="s6")
    nc.vector.tensor_add(s6, v6[:, :, :, 0, :], v6[:, :, :, 1, :])
    nc.vector.tensor_add(s6, s6, v6[:, :, :, 2, :])
    nc.vector.tensor_add(s6, s6, v6[:, :, :, 3, :])
    with lp(reason="pyramid means"):
        nc.vector.tensor_scalar_mul(
            pf[:, :, 10:46], s6.rearrange("p k a b -> p k (a b)"), 1.0 / 256.0
        )
    # ---- scale 3 (2x2 of scale-6 cells = 32x32 px) ----
    c3 = pool.tile([P, ko, 6, 3], f, tag="c3")
    nc.vector.reduce_sum(c3, s6.rearrange("p k a (b bi) -> p k a b bi", bi=2), axis=AX.X)
    v3 = c3.rearrange("p k (a ai) b -> p k a ai b", ai=2)
    s3 = pool.tile([P, ko, 3, 3], f, tag="s3")
    nc.vector.tensor_add(s3, v3[:, :, :, 0, :], v3[:, :, :, 1, :])
    with lp(reason="pyramid means"):
        nc.vector.tensor_scalar_mul(
            pf[:, :, 1:10], s3.rearrange("p k a b -> p k (a b)"), 1.0 / 1024.0
        )
    # ---- scale 1 ----
    t1 = pool.tile([P, ko, 1], f, tag="t1")
    nc.vector.reduce_sum(t1, s3.rearrange("p k a b -> p k (a b)"), axis=AX.X)
    with lp(reason="pyramid means"):
        nc.vector.tensor_scalar_mul(pf[:, :, 0:1], t1, 1.0 / 9216.0)
    # ---- scale 8 (cells of 3x3 grid entries = 12x12 px) ----
    c8 = pool.tile([P, ko, 24, 8], BF16, tag="c8")
    with lp(reason="pool partials"):
        nc.vector.reduce_sum(c8, g24.rearrange("p k a (b bi) -> p k a b bi", bi=3), axis=AX.X)
    v8 = c8.rearrange("p k (a ai) b -> p k a ai b", ai=3)
    s8 = pool.tile([P, ko, 8, 8], f, tag="s8")
    nc.vector.tensor_add(s8, v8[:, :, :, 0, :], v8[:, :, :, 1, :])
    nc.vector.tensor_add(s8, s8, v8[:, :, :, 2, :])
    with lp(reason="pyramid means"):
        nc.vector.tensor_scalar_mul(
            pf[:, :, 46:110], s8.rearrange("p k a b -> p k (a b)"), 1.0 / 144.0
        )


def _build_body(ctx: ExitStack, tc: tile.TileContext, x_d, x8_d, wk8_d, wvt_d,
                wct_d, bwt_d, bk_d, bf_d, out_d):
    nc = tc.nc

    consts = ctx.enter_context(tc.tile_pool(name="consts", bufs=1))
    big = ctx.enter_context(tc.tile_pool(name="big", bufs=1))
    stage = ctx.enter_context(tc.tile_pool(name="stage", bufs=2))
    poolb = ctx.enter_context(tc.tile_pool(name="poolb", bufs=1))
    work = ctx.enter_context(tc.tile_pool(name="work", bufs=6))
    outp = ctx.enter_context(tc.tile_pool(name="outp", bufs=3))

    # ---- phase-1-critical weights first so the first matmul isn't stuck
    # behind phase-2/3 constants on the DMA queue (all weight params are
    # host-packed [P, ...] so each partition is one contiguous descriptor) ----
    wk8 = consts.tile([P, 4, CK], FP8)
    nc.sync.dma_start(wk8, wk8_d[:])
    bkb = consts.tile([P, 2], F32)
    nc.sync.dma_start(bkb, bk_d[:])
    bwt = consts.tile([P, 4, COUT], BF16)

    # ---- persistent full-res activations ----
    kfb = big.tile([P, 2, N], FP8)     # relu(key/query features)
    outb = big.tile([P, 4, N], BF16)   # Bw' @ X partial of the output conv
    gcolx = poolb.tile([P, 4, 2304], BF16)  # X col-pool sums (4-px groups)
    g24k = poolb.tile([P, 2, 24, 24], F32)  # KF 4x4-block sums (24x24 grid)

    xv = x_d[:].rearrange("(kc p) n -> p kc n", p=P)
    x8v = x8_d[:].rearrange("(kc p) n -> p kc n", p=P)
    ov = out_d[:].rearrange("(mc p) n -> p mc n", p=P)
    kg = kfb.rearrange("p k (h w) -> p k h w", w=W)
    hc_done = 0

    # ---- phase 1: stream X in blocks of NB tiles; KF (fp8 DoubleRow),
    # OUTB (bf16), col-pools. Stationary-major loop order so one LDWEIGHTS
    # serves NB matmuls. ----
    nblocks = NTILES // NB
    # phase-2/3 consts, declared early so their DMAs overlap phase-1 compute
    # (issued after the first block's X DMAs below)
    wvt = consts.tile([P, 4, CV], BF16)
    wct = consts.tile([P, 2, COUT], BF16)
    bfb = consts.tile([P, 4], F32)
    g24x = poolb.tile([P, 4, 24, 24], BF16)
    gxv = gcolx.rearrange("p k (hb hi wb) -> p k hb hi wb", hi=4, wb=24)
    def kf_stage(psA, b, x8t):
        # KF: fp8 DoubleRow, contraction 512 = 2 k-pair groups
        c0 = b * NB * NT
        for mc in range(2):
            ps = psA.tile([P, NB, NT], F32, tag="kf", name=f"kps{b}_{mc}")
            for k2 in range(2):
                for j in range(NB):
                    nc.tensor.matmul(ps[:, j, :],
                                     wk8[:, 2 * k2:2 * k2 + 2, ts(mc, P)],
                                     x8t[:, 2 * k2:2 * k2 + 2, ts(j, NT)],
                                     start=(k2 == 0), stop=(k2 == 1),
                                     perf_mode=DR)
            nc.scalar.activation(kfb[:, mc, ds(c0, NB * NT)],
                                 ps.rearrange("p j n -> p (j n)"), RELU,
                                 bias=bkb[:, mc:mc + 1], scale=1.0 / WKSCALE)
        # KF pooling straight to the 24x24 grid, in 12-row chunks (kfb is
        # persistent, so chunks can span tile boundaries)
        nonlocal hc_done
        while hc_done < 8 and (hc_done + 1) * 1152 <= c0 + NB * NT:
            hc = hc_done
            for k in range(2):
                src_ap = kg[:, k, ts(hc, 12), :].rearrange(
                    "p (hb hi) (wb wi) -> p hb wb hi wi", hi=4, wi=4)
                nc.vector.reduce_sum(g24k[:, k, ts(hc, 3), :], src_ap,
                                     axis=AX.XY)
            hc_done += 1

    def outb_stage(psA, b, xt):
        # OUTB: bf16, mc 0..3
        c0 = b * NB * NT
        for mc in range(4):
            ps = psA.tile([P, NB, NT], F32, tag="ob", name=f"obs{b}_{mc}")
            for kc in range(4):
                for j in range(NB):
                    nc.tensor.matmul(ps[:, j, :], bwt[:, kc, ts(mc, P)],
                                     xt[:, kc, ts(j, NT)],
                                     start=(kc == 0), stop=(kc == 3))
            nc.scalar.activation(outb[:, mc, ds(c0, NB * NT)],
                                 ps.rearrange("p j n -> p (j n)"), COPY)

    def xpool_stage(b, xt):
        # X col-pool for this block (4-px groups along w). Stage 1 is a
        # packed pair-add on DVE (16-bit 2x mode: (x0+x2, x1+x3)); stage 2
        # adds the interleaved pair on idle GpSimd.
        ng = NB * P  # 4-px groups in this block
        xg = xt.rearrange("p k (g gi) -> p k g gi", gi=4)
        gct = stage.tile([P, 4, ng, 2], BF16, tag="gct", name=f"gct{b}")
        with nc.allow_low_precision(reason="pyramid-pool partials in bf16"):
            nc.vector.tensor_add(gct, xg[:, :, :, 0:2], xg[:, :, :, 2:4])
            nc.gpsimd.tensor_add(gcolx[:, :, ds(b * ng, ng)],
                                 gct[:, :, :, 0], gct[:, :, :, 1])
        # X row-pool onto the 24x24 grid, in 8-row-of-grid chunks
        # (8 grid rows = 32 px rows = 3072 px = 3 blocks)
        if b % 3 == 2:
            hb0 = (b // 3) * 8
            sl = (slice(None), slice(None), ds(hb0, 8))
            nc.vector.tensor_add(g24x[sl], gxv[sl + (0,)], gxv[sl + (1,)])
            nc.vector.tensor_add(g24x[sl], g24x[sl], gxv[sl + (2,)])
            nc.vector.tensor_add(g24x[sl], g24x[sl], gxv[sl + (3,)])

    with tc.tile_pool(name="psA", bufs=2, space="PSUM") as psA:
        for b in range(nblocks):
            c0 = b * NB * NT
            x8t = stage.tile([P, 4, NB * NT], FP8, tag="x8t", name=f"x8t{b}")
            for j in range(NB):
                nc.sync.dma_start(x8t[:, :, ts(j, NT)], x8v[:, :, ds(c0 + j * NT, NT)])
            if b == 0:
                nc.sync.dma_start(bwt, bwt_d[:])
            xt = stage.tile([P, 4, NB * NT], BF16, tag="xt", name=f"xt{b}")
            for j in range(NB):
                nc.sync.dma_start(xt[:, :, ts(j, NT)], xv[:, :, ds(c0 + j * NT, NT)])
            if b == 1:
                nc.sync.dma_start(wvt, wvt_d[:])
                nc.sync.dma_start(wct, wct_d[:])
                nc.sync.dma_start(bfb, bf_d[:])
            kf_stage(psA, b, x8t)
            outb_stage(psA, b, xt)
            if b < nblocks - 1:
                xpool_stage(b, xt)
        last_xt = xt

    # ---- small on-chip constants ----
    ones_sxs = consts.tile([S, S], BF16)
    nc.vector.memset(ones_sxs, 1.0)
    ident = consts.tile([P, P], BF16)
    make_identity(nc, ident)

    with tc.tile_pool(name="psB", bufs=2, space="PSUM") as psB, \
         tc.tile_pool(name="psO", bufs=4, space="PSUM") as psO:
        # ---- phase 2, critical half: KF pyramid (gates the first SIM).
        # The X-pool pyramid + VT^T/WV2 chain is deferred into the pair loop
        # below -- WV2 isn't consumed until pair 0's output stage (lag 2). ----
        kpx = consts.tile([P, 2, SP], FP8)
        _psp_stage2(nc, poolb, g24k, 2, kpx)

        pfx = consts.tile([P, 4, S], BF16)
        vtT = consts.tile([P, 2, S], BF16)
        wv2 = consts.tile([S, COUT], BF16)

        def phase2_deferred():
            _psp_stage2(nc, poolb, g24x, 4, pfx)
            # VT^T = Wv @ PFX : [2*128 (v), 110 (s)]
            for vc in range(2):
                vtT_ps = psB.tile([P, S], F32, tag="sim", name=f"vtT_ps{vc}")
                for kc in range(4):
                    nc.tensor.matmul(vtT_ps, wvt[:, kc, ts(vc, P)], pfx[:, kc, :],
                                     start=(kc == 0), stop=(kc == 3))
                nc.scalar.copy(vtT[:, vc, :], vtT_ps)
            # WV2 = VT @ Wc^T : [110 (s), 512 (out)] -- stationary for the
            # attention->output matmul (folds the value->out conv into VT)
            wv2_ps = psB.tile([P, NT], F32, tag="csb", name="wv2_ps")
            for vc in range(2):
                nc.tensor.matmul(wv2_ps[:S, :], vtT[:, vc, :], wct[:, vc, :],
                                 start=(vc == 0), stop=(vc == 1))
            nc.scalar.copy(wv2, wv2_ps[:S, :])

        # ---- phase 3: attention + output, software-pipelined over pairs of
        # N tiles with lag 2: pair k's softmax chain (ACT exp -> PE colsum ->
        # DVE recip -> GpSimd mult) fills while pair k-2's output matmuls
        # stream. Output PSUM groups are 1 bank each (bufs=4) so drains
        # release banks at a fine grain. ----
        def out_stage(tt, ens):
            for mc in range(4):
                ops = []
                for tloc in range(2):
                    op = psO.tile([P, NT], F32, tag="outp",
                                  name=f"ops{tt[0]}_{mc}_{tloc}")
                    nc.tensor.matmul(op, ident, outb[:, mc, ts(tt[tloc], NT)],
                                     start=True, stop=False)
                    ops.append(op)
                for tloc in range(2):
                    nc.tensor.matmul(ops[tloc], wv2[:, ts(mc, P)],
                                     ens[tloc][:S, :], start=False, stop=True)
                osb = outp.tile([P, 2, NT], BF16, tag="osb")
                for tloc in range(2):
                    osv = osb[:, tloc, :]
                    if (mc + tloc) % 2 == 0:
                        nc.scalar.activation(osv, ops[tloc], RELU,
                                             bias=bfb[:, mc:mc + 1])
                    else:
                        with nc.allow_low_precision(reason="output bf16"):
                            nc.vector.tensor_scalar(osv, ops[tloc],
                                                    scalar1=bfb[:, mc:mc + 1],
                                                    scalar2=0.0, op0=ADD, op1=MAX)
                nc.sync.dma_start(ov[:, mc, ds(tt[0] * NT, 2 * NT)],
                                  osb.rearrange("p j n -> p (j n)"))

        pending = []
        for tp in range(NTILES // 2):
            tt = (2 * tp, 2 * tp + 1)
            sims = []
            for t in tt:
                sim_ps = psB.tile([P, NT], F32, tag="sim")
                nc.tensor.matmul(sim_ps[:S, :], kpx[:, 0:2, 0:S],
                                 kfb[:, 0:2, ds(t * NT, NT)],
                                 start=True, stop=True, perf_mode=DR)
                sims.append(sim_ps)
            e1s = []
            for sim_ps in sims:
                e1 = work.tile([P, NT], BF16, tag="e1")
                nc.scalar.activation(e1[:S, :], sim_ps[:S, :], EXP, scale=0.0625)
                e1s.append(e1)
            if tp == 0:
                xpool_stage(nblocks - 1, last_xt)
            elif tp == 1:
                phase2_deferred()
            if len(pending) >= 2:
                out_stage(*pending.pop(0))
            ens = []
            for e1 in e1s:
                # colsum broadcast to all 110 partitions in one matmul
                csb_ps = psB.tile([P, NT], F32, tag="csb")
                nc.tensor.matmul(csb_ps[:S, :], ones_sxs, e1[:S, :],
                                 start=True, stop=True)
                rcol = work.tile([P, NT], F32, tag="rcol")
                nc.vector.reciprocal_approx_fast(rcol[:S, :], csb_ps[:S, :])
                en = work.tile([P, NT], BF16, tag="en")
                with nc.allow_low_precision(reason="softmax norm"):
                    nc.gpsimd.tensor_mul(en[:S, :], e1[:S, :], rcol[:S, :])
                ens.append(en)
            pending.append((tt, ens))
        for item in pending:
            out_stage(*item)


def _prune_redundant_ldweights(nc):
    """Remove back-to-back InstLdweights that reload the exact same stationary
    operand (walrus emits one LDWEIGHTS per matmul; our stationary-major loop
    order makes most of them redundant, and dropping them lets consecutive
    matmuls pipeline their fill/drain). All stationaries in this kernel are
    written exactly once before first use, so a signature match is sufficient.
    """
    n_pruned = 0
    for f in nc.m.functions:
        for blk in f.blocks:
            insts = list(blk.instructions)
            out = []
            last_sig = None
            for inst in insts:
                tname = type(inst).__name__
                eng = getattr(inst, "engine", None)
                if eng == mybir.EngineType.PE:
                    if tname == "InstLdweights":
                        ap = inst.ins[0]
                        sig = (ap.memref, ap.offset, str(ap.ap), str(ap.dtype),
                               str(inst.perf_mode), str(inst.is_transpose),
                               str(inst.tile_position), str(inst.tile_size))
                        if sig == last_sig and inst.sync_info is None:
                            n_pruned += 1
                            continue
                        last_sig = sig
                    elif tname in ("InstMatmult", "InstEventSemaphore"):
                        pass
                    else:
                        last_sig = None
                out.append(inst)
            blk.instructions = out
    return n_pruned


def _patch_act_tables():
    """Force every activation onto the one table that holds Exp, Relu and
    Copy together (`natural_log_exp_and_others`), so the kernel does a single
    ACT_TABLE_LOAD instead of reloading on table switches.

    Table ids are positional (index into act_info.json), so we keep the dict
    order/size and just empty the other entries.
    """
    import concourse.hw_specs as hw_specs

    if getattr(bacc, "_apnb_act_patch", False):
        return
    orig = hw_specs.get_activation_tables

    def patched(module_arch):
        tabs = orig(module_arch)
        keep = "natural_log_exp_and_others"
        if keep not in tabs:
            return tabs
        return {k: (v if k == keep else set()) for k, v in tabs.items()}

    bacc.get_activation_tables = patched
    bacc._apnb_act_patch = True


def build_nc():
    _patch_act_tables()
    nc = bacc.Bacc("TRN2", target_bir_lowering=False, debug=False)
    x_d = nc.declare_dram_parameter("x", [CIN, N], BF16, isOutput=False)
    x8_d = nc.declare_dram_parameter("x8", [CIN, N], FP8, isOutput=False)
    wk8_d = nc.declare_dram_parameter("wk8", [P, 4 * CK], FP8, isOutput=False)
    wvt_d = nc.declare_dram_parameter("wvt", [P, 4 * CV], BF16, isOutput=False)
    wct_d = nc.declare_dram_parameter("wct", [P, 2 * COUT], BF16, isOutput=False)
    bwt_d = nc.declare_dram_parameter("bwt", [P, 4 * COUT], BF16, isOutput=False)
    bk_d = nc.declare_dram_parameter("bk", [P, 2], F32, isOutput=False)
    bf_d = nc.declare_dram_parameter("bf", [P, 4], F32, isOutput=False)
    out_d = nc.declare_dram_parameter("out", [COUT, N], BF16, isOutput=True)
    with tile.TileContext(nc) as tc:
        with ExitStack() as ctx:
            _build_body(ctx, tc, x_d, x8_d, wk8_d, wvt_d, wct_d, bwt_d, bk_d,
                        bf_d, out_d)
    nc.compile()
    _prune_redundant_ldweights(nc)
    return nc


def _get_nc():
    global _NC
    if _NC is None:
        _NC = build_nc()
    return _NC


def fold_params(Wk, bk, gk, betak, mk, vk, Wv, bv, Ww, bw, Wo, bo, go, betao,
                mo, vo):
    """Fold BN params + the Ww conv into effective weights (all f32 numpy)."""
    bf16 = ml_dtypes.bfloat16
    fp8 = ml_dtypes.float8_e4m3
    sk = gk / np.sqrt(vk + EPS)
    Wk_f = sk[:, None] * Wk
    bk_f = (bk - mk) * sk + betak
    so = go / np.sqrt(vo + EPS)
    A = so[:, None] * Wo[:, :CIN]      # applies to ctx2 = Ww@ctx + bw
    Bw = so[:, None] * Wo[:, CIN:]     # applies to feats
    b0 = (bo - mo) * so + betao
    Wc = A @ Ww                        # (COUT, CV)
    # attn rows sum to 1  =>  value bias bv contributes Wc @ bv everywhere
    bf_ = b0 + A @ bw + Wc @ bv
    def pack(Wt):
        # [kc*128+p, m] -> [p, kc*M+m]: one contiguous DMA row per partition
        kc = Wt.shape[0] // P
        return np.ascontiguousarray(
            Wt.reshape(kc, P, -1).transpose(1, 0, 2).reshape(P, -1))

    return {
        "wk8": pack(Wk_f.T * WKSCALE).astype(fp8),
        "wvt": pack(Wv.T).astype(bf16),
        "wct": pack(Wc.T).astype(bf16),
        "bwt": pack(Bw.T).astype(bf16),
        "bk": pack(bk_f[:, None]).astype(np.float32),
        "bf": pack(bf_[:, None]).astype(np.float32),
    }


def kernel(**inputs):
    global LAST_RESULT
    feats = np.asarray(inputs["feats"], np.float32)
    B = feats.shape[0]
    assert feats.shape == (B, CIN, H, W) and B == N_CORES

    common = fold_params(
        np.asarray(inputs["Wk"], np.float32), np.asarray(inputs["bk"], np.float32),
        np.asarray(inputs["gk"], np.float32), np.asarray(inputs["betak"], np.float32),
        np.asarray(inputs["mk"], np.float32), np.asarray(inputs["vk"], np.float32),
        np.asarray(inputs["Wv"], np.float32), np.asarray(inputs["bv"], np.float32),
        np.asarray(inputs["Ww"], np.float32), np.asarray(inputs["bw"], np.float32),
        np.asarray(inputs["Wo"], np.float32), np.asarray(inputs["bo"], np.float32),
        np.asarray(inputs["go"], np.float32), np.asarray(inputs["betao"], np.float32),
        np.asarray(inputs["mo"], np.float32), np.asarray(inputs["vo"], np.float32),
    )
    bf16 = ml_dtypes.bfloat16
    fp8 = ml_dtypes.float8_e4m3
    in_maps = []
    for i in range(N_CORES):
        xi = np.ascontiguousarray(feats[i].reshape(CIN, N))
        in_maps.append({"x": xi.astype(bf16), "x8": xi.astype(fp8), **common})
    nc = _get_nc()
    res = run_bass_kernel_spmd(nc, in_maps, core_ids=list(range(N_CORES)),
                               trace=PROFILE)
    LAST_RESULT = res
    out = np.stack([res.results[i]["out"].astype(np.float32).reshape(COUT, H, W)
                    for i in range(N_CORES)])
    return out
